# revision 1
# baseline (speedup 1.0000x reference)
"""MiniGPT forward pass on 8 Trainium2 NeuronCores (Bass/Tile SPMD kernel).

Model: V=32000, T=2048, D=512, H=8 heads, L=4 layers, DFF=2048, B=2, S=2048.

Sharding (8 cores, one SPMD program):
- Tokens: core c owns 512 tokens = flat[512c : 512c+512] (batch c//4).
- Attention: head-parallel within each batch group of 4 cores; core c computes
  heads (2*(c%4), 2*(c%4)+1) over its batch's full 2048 tokens. QKV and
  attention outputs are redistributed with AllToAll over the batch group.
- LM head: vocab-parallel; core c computes logits[:, 4000c:4000c+4000] for all
  4096 tokens after an AllGather of the final hidden states.

Layouts: residual h is [token, feature] fp32 in SBUF. LN outputs are cast to
bf16 and PE-transposed to [feature, token] as matmul operands. LN gamma/beta
are folded into the following matmul weights on the host.
"""
import sys

sys.path.insert(0, "/opt/trn_rl_repo")

import numpy as np
import ml_dtypes

import concourse.bass as bass
import concourse.mybir as mybir
import concourse.tile as tile
from concourse import bacc, bass_utils

BF16 = mybir.dt.bfloat16
F32 = mybir.dt.float32
I32 = mybir.dt.int32
AF = mybir.ActivationFunctionType
OP = mybir.AluOpType

V, T, D, H, L = 32000, 2048, 512, 8, 4
HD = D // H          # 64
DFF = 4 * D          # 2048
B, S = 2, 2048
NC = 8               # cores
TOK = 512            # tokens per core
VSH = V // NC        # 4000 vocab per core
NEG = -1.0e9


def build_nc():
    nc = bacc.Bacc("TRN2", target_bir_lowering=False, debug=False, num_devices=NC)

    # ---- I/O ----
    tok_emb = nc.dram_tensor("tok_emb", [V, D], F32, kind="ExternalInput")
    pos = nc.dram_tensor("pos", [TOK, D], F32, kind="ExternalInput")
    xidx = nc.dram_tensor("xidx", [TOK, 1], I32, kind="ExternalInput")
    wqkvT = nc.dram_tensor("wqkvT", [L, D, 3 * D], BF16, kind="ExternalInput")
    bqkv = nc.dram_tensor("bqkv", [L * 12 * 128, 1], F32, kind="ExternalInput")
    wprojT = nc.dram_tensor("wprojT", [L, D, D], BF16, kind="ExternalInput")
    bproj = nc.dram_tensor("bproj", [L, 1, D], F32, kind="ExternalInput")
    wffn1T = nc.dram_tensor("wffn1T", [L, D, DFF], BF16, kind="ExternalInput")
    bffn1 = nc.dram_tensor("bffn1", [L * 16 * 128, 1], F32, kind="ExternalInput")
    wffn2T = nc.dram_tensor("wffn2T", [L, DFF, D], BF16, kind="ExternalInput")
    bffn2 = nc.dram_tensor("bffn2", [L, 1, D], F32, kind="ExternalInput")
    lmT = nc.dram_tensor("lmT", [D, VSH], BF16, kind="ExternalInput")
    lmb = nc.dram_tensor("lmb", [1, VSH], F32, kind="ExternalInput")
    mstrip = nc.dram_tensor("mstrip", [128, 896], BF16, kind="ExternalInput")
    ident_in = nc.dram_tensor("ident_in", [128, 128], BF16, kind="ExternalInput")
    ones_in = nc.dram_tensor("ones_in", [1, 128], BF16, kind="ExternalInput")
    logits = nc.dram_tensor("logits", [B * S, VSH], F32, kind="ExternalOutput")

    lmidx = nc.dram_tensor("lmidx", [128, 144], I32, kind="ExternalInput")

    # ---- internal DRAM (collective bounces) ----
    kv_ai = [nc.dram_tensor(f"kv_ai{l}", [2 * D, TOK], BF16) for l in range(L)]
    kv_ao = [nc.dram_tensor(f"kv_ao{l}", [2 * D, TOK], BF16) for l in range(L)]
    q_ai = [nc.dram_tensor(f"q_ai{l}", [D, TOK], BF16) for l in range(L)]
    q_ao = [nc.dram_tensor(f"q_ao{l}", [D, TOK], BF16) for l in range(L)]
    att_ai = [nc.dram_tensor(f"att_ai{l}", [D, TOK], BF16) for l in range(L)]
    att_ao = [nc.dram_tensor(f"att_ao{l}", [D, TOK], BF16) for l in range(L)]
    ag_in = nc.dram_tensor("ag_in", [D, TOK], BF16)
    ag_out = nc.dram_tensor("ag_out", [NC * D, TOK], BF16, addr_space="Shared")
    grp = [list(range(NC))]

    with tile.TileContext(nc) as tc:
        with (
            tc.tile_pool(name="const", bufs=1) as cp,
            tc.tile_pool(name="persist", bufs=1) as pp,
        ):
            ident = cp.tile([128, 128], BF16, name="ident")
            ones_r = cp.tile([1, 128], BF16, name="ones_r")
            msk = cp.tile([128, 896], BF16, name="msk")
            projb_bc = cp.tile([128, L * D], BF16, name="projb_bc")
            ffn2b_bc = cp.tile([128, L * D], BF16, name="ffn2b_bc")
            lmb_bc = cp.tile([128, VSH], BF16, name="lmb_bc")
            brow = cp.tile([1, 512], F32, name="brow")
            brow_bf = cp.tile([1, 512], BF16, name="brow_bf")
            eps_t = cp.tile([128, 1], F32, name="eps_t")
            hts = [pp.tile([128, D], F32, name=f"h{t}") for t in range(4)]
            idx_sb = pp.tile([128, 4], I32, name="idx_sb")
            lmidx_sb = pp.tile([128, 144], I32, name="lmidx_sb")
            vones = pp.tile([128, 32 * 65], BF16, name="vones")

            with (
                tc.tile_pool(name="wpool", bufs=1) as wp,
                tc.tile_pool(name="work", bufs=2) as wk,
                tc.tile_pool(name="exppool", bufs=3) as ep,
                tc.tile_pool(name="pmm", bufs=2, space="PSUM") as pmm,
                tc.tile_pool(name="psc", bufs=2, space="PSUM") as psc,
                tc.tile_pool(name="pout", bufs=2, space="PSUM") as pout,
            ):
                # ================= prologue =================
                nc.sync.dma_start(out=ident[:], in_=ident_in[:])
                nc.sync.dma_start(out=ones_r[:], in_=ones_in[:])
                nc.sync.dma_start(out=msk[:], in_=mstrip[:])
                nc.vector.memset(eps_t[:], 1e-5)

                def bcast_row(dst_ap, src_dram_ap, n):
                    # dst[128, n] = broadcast of src[1, n] across partitions
                    done = 0
                    while done < n:
                        w = min(512, n - done)
                        nc.sync.dma_start(out=brow[:, :w], in_=src_dram_ap[:, done:done + w])
                        nc.vector.tensor_copy(out=brow_bf[:, :w], in_=brow[:, :w])
                        ps = pmm.tile([128, 512], F32, tag="pmm")
                        nc.tensor.matmul(ps[:, :w], lhsT=ones_r[:, :], rhs=brow_bf[:, :w],
                                         start=True, stop=True)
                        nc.vector.tensor_copy(out=dst_ap[:, done:done + w], in_=ps[:, :w])
                        done += w

                for l in range(L):
                    bcast_row(projb_bc[:, l * D:(l + 1) * D], bproj[l], D)
                    bcast_row(ffn2b_bc[:, l * D:(l + 1) * D], bffn2[l], D)
                bcast_row(lmb_bc[:, :], lmb[:, :], VSH)

                # embeddings -> residual h [128 tok, 4 blocks * 512 feat] fp32
                for t in range(4):
                    nc.sync.dma_start(out=idx_sb[:, t:t + 1], in_=xidx[128 * t:128 * (t + 1), :])
                nc.sync.dma_start(out=lmidx_sb[:], in_=lmidx[:])
                for t in range(4):
                    emb = wk.tile([128, D], F32, tag="emb", bufs=1)
                    nc.gpsimd.indirect_dma_start(
                        out=emb[:], out_offset=None, in_=tok_emb[:],
                        in_offset=bass.IndirectOffsetOnAxis(ap=idx_sb[:, t:t + 1], axis=0),
                    )
                    pos_t = wk.tile([128, D], F32, tag="emb2", bufs=1)
                    nc.sync.dma_start(out=pos_t[:], in_=pos[128 * t:128 * (t + 1), :])
                    nc.vector.tensor_tensor(out=hts[t][:], in0=emb[:], in1=pos_t[:], op=OP.add)

                nc.vector.memset(vones[:], 1.0)

                # ---- helpers ----
                def layernorm_T(srcs, dst_bf_T, eps=1e-5):
                    """srcs: 4 tiles [128, D] fp32 [tok, feat]. Writes dst_bf_T
                    [128, 4*512] bf16 = transposed ([feat-ptile, tok]) normalized."""
                    hln = wk.tile([128, 4 * D], BF16, tag="hln", bufs=1)
                    nmu4 = wk.tile([128, 4], F32, tag="lnmu")
                    var4 = wk.tile([128, 4], F32, tag="lnvar")
                    rs4 = wk.tile([128, 4], F32, tag="lnrs")
                    for t in range(4):
                        s = wk.tile([128, 1], F32, tag="lns")
                        ssq = wk.tile([128, 1], F32, tag="lns")
                        sq = wk.tile([128, D], F32, tag="lnsq", bufs=1)
                        blk = srcs[t][:]
                        nc.vector.tensor_reduce(out=s[:], in_=blk, axis=mybir.AxisListType.X, op=OP.add)
                        nc.vector.tensor_tensor(out=sq[:], in0=blk, in1=blk, op=OP.mult)
                        nc.vector.tensor_reduce(out=ssq[:], in_=sq[:], axis=mybir.AxisListType.X, op=OP.add)
                        nc.vector.tensor_scalar_mul(nmu4[:, t:t + 1], s[:], -1.0 / D)
                        mu2 = wk.tile([128, 1], F32, tag="lns")
                        nc.vector.tensor_tensor(out=mu2[:], in0=nmu4[:, t:t + 1], in1=nmu4[:, t:t + 1], op=OP.mult)
                        nc.vector.scalar_tensor_tensor(out=var4[:, t:t + 1], in0=ssq[:], scalar=1.0 / D,
                                                       in1=mu2[:], op0=OP.mult, op1=OP.subtract)
                    nc.scalar.activation(out=rs4[:], in_=var4[:], func=AF.Ln, bias=eps_t[:])
                    nc.scalar.activation(out=rs4[:], in_=rs4[:], func=AF.Exp, scale=-0.5)
                    for t in range(4):
                        nc.vector.tensor_scalar(out=hln[:, D * t:D * (t + 1)], in0=srcs[t][:],
                                                scalar1=nmu4[:, t:t + 1], scalar2=rs4[:, t:t + 1],
                                                op0=OP.add, op1=OP.mult)
                    for f in range(4):
                        tp = pmm.tile([128, 512], BF16, tag="pmm")
                        for t in range(4):
                            nc.tensor.transpose(out=tp[:, 128 * t:128 * (t + 1)],
                                                in_=hln[:, D * t + 128 * f: D * t + 128 * (f + 1)],
                                                identity=ident[:])
                        nc.vector.tensor_copy(out=dst_bf_T[:, 512 * f:512 * (f + 1)], in_=tp[:])

                # ================= transformer layers =================
                for l in range(L):
                    wq = wp.tile([128, 4 * 1536], BF16, tag="wq")
                    nc.sync.dma_start(out=wq[:].rearrange("p (c e) -> p c e", c=4),
                                      in_=wqkvT[l].rearrange("(c p) e -> p c e", p=128))
                    wpj = wp.tile([128, 4 * 512], BF16, tag="wpj")
                    nc.sync.dma_start(out=wpj[:].rearrange("p (c e) -> p c e", c=4),
                                      in_=wprojT[l].rearrange("(c p) e -> p c e", p=128))
                    wf1 = wp.tile([128, 4 * 2048], BF16, tag="wf1")
                    nc.sync.dma_start(out=wf1[:].rearrange("p (c e) -> p c e", c=4),
                                      in_=wffn1T[l].rearrange("(c p) e -> p c e", p=128))
                    wf2 = wp.tile([128, 16 * 512], BF16, tag="wf2")
                    nc.sync.dma_start(out=wf2[:].rearrange("p (c e) -> p c e", c=16),
                                      in_=wffn2T[l].rearrange("(c p) e -> p c e", p=128))
                    bq = wp.tile([128, 12], F32, tag="bq")
                    for o in range(12):
                        nc.sync.dma_start(out=bq[:, o:o + 1],
                                          in_=bqkv[(l * 12 + o) * 128:(l * 12 + o + 1) * 128, :])
                    bf1 = wp.tile([128, 16], F32, tag="bf1")
                    for o in range(16):
                        nc.sync.dma_start(out=bf1[:, o:o + 1],
                                          in_=bffn1[(l * 16 + o) * 128:(l * 16 + o + 1) * 128, :])

                    # -- LN1 + transpose --
                    hlnT = wk.tile([128, 4 * 512], BF16, tag="hlnT", bufs=1)
                    layernorm_T(hts, hlnT)

                    # -- qkvT = W' @ hlnT ([3D feat, 512 tok]); k,v first so the
                    # kv AllToAll overlaps the q matmuls --
                    qkvT = wk.tile([128, 12 * 512], BF16, tag="qkvT", bufs=1)
                    for o in [4, 5, 6, 7, 8, 9, 10, 11, 0, 1, 2, 3]:
                        ps = pmm.tile([128, 512], F32, tag="pmm")
                        for kc in range(4):
                            nc.tensor.matmul(ps[:],
                                             lhsT=wq[:, 1536 * kc + 128 * o:1536 * kc + 128 * (o + 1)],
                                             rhs=hlnT[:, 512 * kc:512 * (kc + 1)],
                                             start=(kc == 0), stop=(kc == 3))
                        nc.vector.tensor_scalar(out=qkvT[:, 512 * o:512 * (o + 1)], in0=ps[:],
                                                scalar1=bq[:, o:o + 1], scalar2=None, op0=OP.add)
                        if o == 11:
                            # kv shard s rows [128s,+128) = head-s k then v of my tokens
                            for s_ in range(8):
                                pb = 64 * (s_ % 2)
                                blk = s_ // 2
                                nc.sync.dma_start(out=kv_ai[l][128 * s_:128 * s_ + 64, :],
                                                  in_=qkvT[pb:pb + 64, 512 * (4 + blk):512 * (5 + blk)])
                                nc.sync.dma_start(out=kv_ai[l][128 * s_ + 64:128 * s_ + 128, :],
                                                  in_=qkvT[pb:pb + 64, 512 * (8 + blk):512 * (9 + blk)])
                            nc.gpsimd.collective_compute(
                                "AllToAll", OP.bypass, replica_groups=grp,
                                ins=[kv_ai[l][:]], outs=[kv_ao[l][:]],
                            )
                    for s_ in range(8):
                        pb = 64 * (s_ % 2)
                        blk = s_ // 2
                        nc.sync.dma_start(out=q_ai[l][64 * s_:64 * (s_ + 1), :],
                                          in_=qkvT[pb:pb + 64, 512 * blk:512 * (blk + 1)])
                    nc.gpsimd.collective_compute(
                        "AllToAll", OP.bypass, replica_groups=grp,
                        ins=[q_ai[l][:]], outs=[q_ao[l][:]],
                    )
                    # my head over both batches: batch b in partition half 64b
                    qT = wk.tile([128, 2048], BF16, tag="qT", bufs=1)
                    kT = wk.tile([128, 2048], BF16, tag="kT", bufs=1)
                    vT = wk.tile([128, 2048], BF16, tag="vT", bufs=1)
                    for r in range(8):
                        b_, rr = r // 4, r % 4
                        nc.sync.dma_start(out=kT[64 * b_:64 * b_ + 64, 512 * rr:512 * (rr + 1)],
                                          in_=kv_ao[l][128 * r:128 * r + 64, :])
                        nc.sync.dma_start(out=vT[64 * b_:64 * b_ + 64, 512 * rr:512 * (rr + 1)],
                                          in_=kv_ao[l][128 * r + 64:128 * r + 128, :])
                        nc.sync.dma_start(out=qT[64 * b_:64 * b_ + 64, 512 * rr:512 * (rr + 1)],
                                          in_=q_ao[l][64 * r:64 * (r + 1), :])
                    for b_ in range(2):
                        hb = 64 * b_
                        for i in range(16):
                            tp = pmm.tile([128, 64], BF16, tag="pmm")
                            nc.tensor.transpose(out=tp[:], in_=vT[hb:hb + 64, 128 * i:128 * (i + 1)],
                                                identity=ident[hb:hb + 64, hb:hb + 64])
                            nc.vector.tensor_copy(out=vones[:, 65 * (16 * b_ + i):65 * (16 * b_ + i) + 64],
                                                  in_=tp[:])

                    # -- attention (my head, both batches, causal, q in 1024-pairs) --
                    attnT = wk.tile([64, 4096], BF16, tag="attnT", bufs=1)
                    for b_ in range(2):
                        hb = 64 * b_
                        for p in range(2):
                            outp = pout.tile([65, 1024], F32, tag="pout")
                            for i in range(8 * p + 8):
                                jlmin = max(0, i // 4 - 2 * p)
                                for jl in (0, 1):
                                    if jl < jlmin:
                                        continue
                                    diag = (i // 4 == 2 * p + jl)
                                    sc = psc.tile([128, 512], F32, tag="psc")
                                    nc.tensor.matmul(
                                        sc[:],
                                        lhsT=kT[hb:hb + 64, 128 * i:128 * (i + 1)],
                                        rhs=qT[hb:hb + 64, 1024 * p + 512 * jl:1024 * p + 512 * (jl + 1)],
                                        start=True, stop=not diag)
                                    if diag:
                                        m = i % 4
                                        nc.tensor.matmul(
                                            sc[:], lhsT=ident[:],
                                            rhs=msk[:, 384 - 128 * m:896 - 128 * m],
                                            start=False, stop=True)
                                    ex = ep.tile([128, 512], BF16, tag="ex")
                                    nc.scalar.activation(out=ex[:], in_=sc[:],
                                                         func=AF.Exp, scale=float(HD) ** -0.5)
                                    kmax = 4 * (2 * p + jl) + 3
                                    nc.tensor.matmul(
                                        outp[:, 512 * jl:512 * (jl + 1)],
                                        lhsT=vones[:, 65 * (16 * b_ + i):65 * (16 * b_ + i + 1)],
                                        rhs=ex[:],
                                        start=(i == 0), stop=(i == kmax))
                            # normalize: rows 0..63 /= row 64
                            # (bcast denom via PE, fast-reciprocal on 64 lanes, multiply)
                            dnb = wk.tile([1, 1024], BF16, tag="rcb", bufs=1)
                            nc.vector.tensor_copy(out=dnb[:], in_=outp[64:65, :])
                            for q2 in range(2):
                                bc = psc.tile([64, 512], F32, tag="psc")
                                nc.tensor.matmul(bc[:], lhsT=ones_r[:, 0:64],
                                                 rhs=dnb[:, 512 * q2:512 * (q2 + 1)], start=True, stop=True)
                                rcs = wk.tile([64, 512], F32, tag="bcs", bufs=1)
                                nc.vector.reciprocal_approx_fast(out=rcs[:], in_=bc[:])
                                nc.vector.tensor_tensor(
                                    out=attnT[:, 2048 * b_ + 1024 * p + 512 * q2:2048 * b_ + 1024 * p + 512 * (q2 + 1)],
                                    in0=outp[0:64, 512 * q2:512 * (q2 + 1)], in1=rcs[:], op=OP.mult)
                    # A2A attention outputs back to token owners
                    for s_ in range(8):
                        nc.sync.dma_start(out=att_ai[l][64 * s_:64 * (s_ + 1), :],
                                          in_=attnT[:, 512 * s_:512 * (s_ + 1)])
                    nc.gpsimd.collective_compute(
                        "AllToAll", OP.bypass, replica_groups=grp,
                        ins=[att_ai[l][:]], outs=[att_ao[l][:]],
                    )
                    aT = wk.tile([128, 4 * 512], BF16, tag="aT", bufs=1)
                    for r in range(4):
                        nc.sync.dma_start(out=aT[:, 512 * r:512 * (r + 1)],
                                          in_=att_ao[l][128 * r:128 * (r + 1), :])

                    # -- proj + residual --
                    for t in range(4):
                        ps = pmm.tile([128, 512], F32, tag="pmm")
                        for fc in range(4):
                            nc.tensor.matmul(ps[:],
                                             lhsT=aT[:, 512 * fc + 128 * t:512 * fc + 128 * (t + 1)],
                                             rhs=wpj[:, 512 * fc:512 * (fc + 1)],
                                             start=(fc == 0), stop=(fc == 3))
                        nc.vector.tensor_tensor(out=ps[:], in0=ps[:],
                                                in1=projb_bc[:, D * l:D * (l + 1)], op=OP.add)
                        nc.vector.tensor_tensor(out=hts[t][:], in0=hts[t][:], in1=ps[:], op=OP.add)

                    # -- LN2 + FFN --
                    hln2T = wk.tile([128, 4 * 512], BF16, tag="hlnT", bufs=1)
                    layernorm_T(hts, hln2T)
                    fT = wk.tile([128, 16 * 512], BF16, tag="fT", bufs=1)
                    for o in range(16):
                        ps = pmm.tile([128, 512], F32, tag="pmm")
                        for kc in range(4):
                            nc.tensor.matmul(ps[:],
                                             lhsT=wf1[:, 2048 * kc + 128 * o:2048 * kc + 128 * (o + 1)],
                                             rhs=hln2T[:, 512 * kc:512 * (kc + 1)],
                                             start=(kc == 0), stop=(kc == 3))
                        nc.scalar.activation(out=fT[:, 512 * o:512 * (o + 1)], in_=ps[:],
                                             func=AF.Gelu, bias=bf1[:, o:o + 1])
                    for t in range(4):
                        ps = pmm.tile([128, 512], F32, tag="pmm")
                        for kc in range(16):
                            nc.tensor.matmul(ps[:],
                                             lhsT=fT[:, 512 * kc + 128 * t:512 * kc + 128 * (t + 1)],
                                             rhs=wf2[:, 512 * kc:512 * (kc + 1)],
                                             start=(kc == 0), stop=(kc == 15))
                        nc.vector.tensor_tensor(out=ps[:], in0=ps[:],
                                                in1=ffn2b_bc[:, D * l:D * (l + 1)], op=OP.add)
                        nc.vector.tensor_tensor(out=hts[t][:], in0=hts[t][:], in1=ps[:], op=OP.add)

                # ================= final LN + AllGather =================
                hfT = pp.tile([128, 4 * 512], BF16, name="hfT")
                layernorm_T(hts, hfT)
                for f in range(4):
                    nc.sync.dma_start(out=ag_in[128 * f:128 * (f + 1), :],
                                      in_=hfT[:, 512 * f:512 * (f + 1)])
                nc.gpsimd.collective_compute(
                    "AllGather", OP.bypass, replica_groups=[list(range(NC))],
                    ins=[ag_in[:]], outs=[ag_out[:]],
                )

            # ================= LM head (body pools closed, PSUM free) ========
            with (
                tc.tile_pool(name="lmw", bufs=1) as lw,
                tc.tile_pool(name="lmwork", bufs=3) as lk,
                tc.tile_pool(name="plm", bufs=2, space="PSUM") as plm,
            ):
                lmw = lw.tile([128, 4 * VSH], BF16, name="lmw")
                nc.sync.dma_start(out=lmw[:].rearrange("p (c e) -> p c e", c=4),
                                  in_=lmT[:].rearrange("(c p) e -> p c e", p=128))

                def lm_tile(lhs_slices, out_idx_col):
                    stage = lk.tile([128, VSH], F32, tag="stage")
                    for hf in range(2):
                        ps = plm.tile([128, 2048], F32, tag="plm")
                        for kc in range(4):
                            for vc in range(4):
                                w0 = VSH * kc + 2000 * hf + 500 * vc
                                nc.tensor.matmul(
                                    ps[:, 512 * vc:512 * vc + 500],
                                    lhsT=lhs_slices[kc],
                                    rhs=lmw[:, w0:w0 + 500],
                                    start=(kc == 0), stop=(kc == 3))
                        ps3 = ps[:].rearrange("p (b e) -> p b e", b=4)[:, :, 0:500]
                        st3 = stage[:, 2000 * hf:2000 * (hf + 1)].rearrange("p (b e) -> p b e", b=4)
                        bc3 = lmb_bc[:, 2000 * hf:2000 * (hf + 1)].rearrange("p (b e) -> p b e", b=4)
                        nc.vector.tensor_tensor(out=st3, in0=ps3, in1=bc3, op=OP.add)
                    nc.gpsimd.indirect_dma_start(
                        out=logits[:],
                        out_offset=bass.IndirectOffsetOnAxis(ap=lmidx_sb[:, out_idx_col:out_idx_col + 1], axis=0),
                        in_=stage[:], in_offset=None)

                # 4 local token tiles first (read hfT directly; overlaps the AllGather)
                for u in range(4):
                    lm_tile([hfT[:, 512 * kc + 128 * u:512 * kc + 128 * (u + 1)] for kc in range(4)], u)
                # 28 remote token tiles via indirect gather from ag_out
                for j in range(28):
                    u = j % 4
                    lhs = lk.tile([128, 4 * 128], BF16, tag="lhs")
                    for kc in range(4):
                        nc.gpsimd.indirect_dma_start(
                            out=lhs[:, 128 * kc:128 * (kc + 1)], out_offset=None,
                            in_=ag_out[:],
                            in_offset=bass.IndirectOffsetOnAxis(
                                ap=lmidx_sb[:, 32 + j * 4 + kc:32 + j * 4 + kc + 1], axis=0),
                            element_offset=128 * u,
                        )
                    lm_tile([lhs[:, 128 * kc:128 * (kc + 1)] for kc in range(4)], 4 + j)

    nc.compile()
    return nc


_NC_CACHE = None


def _get_nc():
    global _NC_CACHE
    if _NC_CACHE is None:
        _NC_CACHE = build_nc()
    return _NC_CACHE


def _prep_inputs(inputs):
    bf = ml_dtypes.bfloat16
    tok_emb = np.asarray(inputs["tok_emb"], np.float32)
    pos_emb = np.asarray(inputs["pos_emb"], np.float32)
    x = np.asarray(inputs["x"]).astype(np.int32).reshape(-1)  # [4096] flat

    def eff(w, g, b, wb):
        # fold the preceding layernorm's gamma/beta into w (out,in) and bias
        w = np.asarray(w, np.float32)
        weff = w * np.asarray(g, np.float32)[None, :]
        beff = w @ np.asarray(b, np.float32) + np.asarray(wb, np.float32)
        return weff, beff

    wqkvT = np.zeros((L, D, 3 * D), bf)
    bqkv = np.zeros((L, 12, 128), np.float32)
    wprojT = np.zeros((L, D, D), bf)
    bproj = np.zeros((L, 1, D), np.float32)
    wffn1T = np.zeros((L, D, DFF), bf)
    bffn1 = np.zeros((L, 16, 128), np.float32)
    wffn2T = np.zeros((L, DFF, D), bf)
    bffn2 = np.zeros((L, 1, D), np.float32)
    for l in range(L):
        w, b = eff(inputs["qkv_w"][l], inputs["ln1_g"][l], inputs["ln1_b"][l], inputs["qkv_b"][l])
        wqkvT[l] = w.T.astype(bf)
        bqkv[l] = b.reshape(12, 128)
        wprojT[l] = np.asarray(inputs["proj_w"][l], np.float32).T.astype(bf)
        bproj[l, 0] = np.asarray(inputs["proj_b"][l], np.float32)
        w, b = eff(inputs["ffn1_w"][l], inputs["ln2_g"][l], inputs["ln2_b"][l], inputs["ffn1_b"][l])
        wffn1T[l] = w.T.astype(bf)
        bffn1[l] = b.reshape(16, 128)
        wffn2T[l] = np.asarray(inputs["ffn2_w"][l], np.float32).T.astype(bf)
        bffn2[l, 0] = np.asarray(inputs["ffn2_b"][l], np.float32)
    lmw, lmbf = eff(inputs["lm_w"], inputs["lnf_g"], inputs["lnf_b"], inputs["lm_b"])

    mstrip = np.full((128, 896), NEG, np.float32)
    kk = np.arange(128)[:, None]
    cc = np.arange(896)[None, :]
    mstrip[kk <= cc - 384] = 0.0
    mstrip = mstrip.astype(bf)

    common = dict(tok_emb=tok_emb, wqkvT=wqkvT, bqkv=bqkv.reshape(L * 12 * 128, 1),
                  wprojT=wprojT, bproj=bproj, wffn1T=wffn1T,
                  bffn1=bffn1.reshape(L * 16 * 128, 1), wffn2T=wffn2T, bffn2=bffn2,
                  mstrip=mstrip, ident_in=np.eye(128, dtype=bf),
                  ones_in=np.ones((1, 128), bf))
    in_maps = []
    pvec = np.arange(128, dtype=np.int32)
    for c in range(NC):
        s0 = 512 * (c % 4)
        m = dict(common)
        m["pos"] = pos_emb[s0:s0 + 512]
        m["xidx"] = x[512 * c:512 * (c + 1)].reshape(TOK, 1)
        lmidx = np.zeros((128, 144), np.int32)
        for u in range(4):
            lmidx[:, u] = 512 * c + 128 * u + pvec          # local scatter rows
        rks = [r for r in range(NC) if r != c]
        for j in range(28):
            ri, u = rks[j // 4], j % 4
            lmidx[:, 4 + j] = 512 * ri + 128 * u + pvec     # remote scatter rows
            for kc in range(4):
                lmidx[:, 32 + j * 4 + kc] = 512 * ri + 128 * kc + pvec  # gather rows
        m["lmidx"] = lmidx
        m["lmT"] = np.ascontiguousarray(lmw[VSH * c:VSH * (c + 1)].T.astype(bf))
        m["lmb"] = lmbf[VSH * c:VSH * (c + 1)].reshape(1, VSH).copy()
        in_maps.append(m)
    return in_maps


def run(inputs, trace=False, tmpdir=None):
    nc = _get_nc()
    in_maps = _prep_inputs(inputs)
    res = bass_utils.run_bass_kernel_spmd(nc, in_maps, list(range(NC)), trace=trace, tmpdir=tmpdir)
    full = np.empty((B * S, V), np.float32)
    for c in range(NC):
        full[:, VSH * c:VSH * (c + 1)] = res.results[c]["logits"]
    return full.reshape(B, S, V), res


def kernel(**inputs) -> np.ndarray:
    out, _ = run(inputs)
    return out



# revision 7
# speedup vs baseline: 1.1503x; 1.1503x over previous
"""MiniGPT forward pass on 8 Trainium2 NeuronCores (Bass/Tile SPMD kernel).

Model: V=32000, T=2048, D=512, H=8 heads, L=4 layers, DFF=2048, B=2, S=2048.

Sharding (8 cores, one SPMD program):
- Tokens: core c owns 512 tokens = flat[512c : 512c+512] (batch c//4).
- Attention: head-parallel; core c computes head c for both batches (batch b
  in partition half 64b) over the batch's full 2048 tokens. QKV and attention
  outputs are redistributed with AllToAll over all 8 cores.
- LM head: vocab-parallel; core c computes logits[:, 4000c:4000c+4000] for all
  4096 tokens after an AllGather of the final hidden states. Logit rows are
  written rotated by -512c (so row offsets are core-independent); the host
  un-rotates with np.roll.

Layouts: residual h is [token, feature] fp32 in SBUF. LN outputs are cast to
bf16 and PE-transposed to [feature, token] as matmul operands. LN gamma/beta
are folded into the following matmul weights on the host. proj/ffn2 biases are
added via rank-1 matmuls into PSUM; qkv bias rides the ACT-engine PSUM->SBUF
copy. The causal mask is applied by skipping fully-masked 128-col strips and
multiplying one 128x128 triangular 0/1 mask after exp on the DVE.
"""
import sys

sys.path.insert(0, "/opt/trn_rl_repo")

import numpy as np
import ml_dtypes

import concourse.bass as bass
import concourse.mybir as mybir
import concourse.tile as tile
from concourse import bacc, bass_utils

BF16 = mybir.dt.bfloat16
F32 = mybir.dt.float32
I32 = mybir.dt.int32
AF = mybir.ActivationFunctionType
OP = mybir.AluOpType

V, T, D, H, L = 32000, 2048, 512, 8, 4
HD = D // H          # 64
DFF = 4 * D          # 2048
B, S = 2, 2048
NC = 8               # cores
TOK = 512            # tokens per core
VSH = V // NC        # 4000 vocab per core


def build_nc(zero_lmb: bool):
    nc = bacc.Bacc("TRN2", target_bir_lowering=False, debug=False, num_devices=NC)

    # ---- I/O ----
    tok_emb = nc.dram_tensor("tok_emb", [V, D], F32, kind="ExternalInput")
    pos = nc.dram_tensor("pos", [TOK, D], F32, kind="ExternalInput")
    xidx = nc.dram_tensor("xidx", [TOK, 1], I32, kind="ExternalInput")
    wqkvT = nc.dram_tensor("wqkvT", [L, D, 3 * D], BF16, kind="ExternalInput")
    bqkv = nc.dram_tensor("bqkv", [L * 12 * 128, 1], F32, kind="ExternalInput")
    wprojT = nc.dram_tensor("wprojT", [L, D, D], BF16, kind="ExternalInput")
    bproj = nc.dram_tensor("bproj", [L, 1, D], BF16, kind="ExternalInput")
    wffn1T = nc.dram_tensor("wffn1T", [L, D, DFF], BF16, kind="ExternalInput")
    bffn1 = nc.dram_tensor("bffn1", [L * 16 * 128, 1], F32, kind="ExternalInput")
    wffn2T = nc.dram_tensor("wffn2T", [L, DFF, D], BF16, kind="ExternalInput")
    bffn2 = nc.dram_tensor("bffn2", [L, 1, D], BF16, kind="ExternalInput")
    lmT = nc.dram_tensor("lmT", [D, VSH], BF16, kind="ExternalInput")
    lmb = nc.dram_tensor("lmb", [1, VSH], F32, kind="ExternalInput")
    ident_in = nc.dram_tensor("ident_in", [128, 128], BF16, kind="ExternalInput")
    ones_in = nc.dram_tensor("ones_in", [1, 128], BF16, kind="ExternalInput")
    tri_in = nc.dram_tensor("tri_in", [128, 128], BF16, kind="ExternalInput")
    logits = nc.dram_tensor("logits", [B * S, VSH], F32, kind="ExternalOutput")

    lmidx = nc.dram_tensor("lmidx", [128, 28], I32, kind="ExternalInput")

    # ---- internal DRAM (collective bounces) ----
    kv_ai = [nc.dram_tensor(f"kv_ai{l}", [2 * D, TOK], BF16) for l in range(L)]
    kv_ao = [nc.dram_tensor(f"kv_ao{l}", [2 * D, TOK], BF16) for l in range(L)]
    q_ai = [nc.dram_tensor(f"q_ai{l}", [D, TOK], BF16) for l in range(L)]
    q_ao = [nc.dram_tensor(f"q_ao{l}", [D, TOK], BF16) for l in range(L)]
    att_ai = [nc.dram_tensor(f"att_ai{l}", [D, TOK], BF16) for l in range(L)]
    att_ao = [nc.dram_tensor(f"att_ao{l}", [D, TOK], BF16) for l in range(L)]
    ag_in = nc.dram_tensor("ag_in", [D, TOK], BF16)
    ag_out = nc.dram_tensor("ag_out", [NC * D, TOK], BF16, addr_space="Shared")
    grp = [list(range(NC))]

    with tile.TileContext(nc) as tc:
        with (
            tc.tile_pool(name="const", bufs=1) as cp,
            tc.tile_pool(name="persist", bufs=1) as pp,
        ):
            ident = cp.tile([128, 128], BF16, name="ident")
            ones_r = cp.tile([1, 128], BF16, name="ones_r")
            tri = cp.tile([128, 128], BF16, name="tri")
            eps_t = cp.tile([128, 1], F32, name="eps_t")
            if not zero_lmb:
                lmb_bc = cp.tile([128, VSH], BF16, name="lmb_bc")
                brow = cp.tile([1, 512], F32, name="brow")
                brow_bf = cp.tile([1, 512], BF16, name="brow_bf")
            hts = [pp.tile([128, D], F32, name=f"h{t}") for t in range(4)]
            idx_sb = pp.tile([128, 4], I32, name="idx_sb")
            lmidx_sb = pp.tile([128, 28], I32, name="lmidx_sb")
            vones = pp.tile([128, 32 * 65], BF16, name="vones")
            hfT = pp.tile([128, 4 * 512], BF16, name="hfT")

            with (
                tc.tile_pool(name="wpool", bufs=2) as wp,
                tc.tile_pool(name="work", bufs=2) as wk,
                tc.tile_pool(name="exppool", bufs=3) as ep,
                tc.tile_pool(name="pmm", bufs=2, space="PSUM") as pmm,
                tc.tile_pool(name="psc", bufs=2, space="PSUM") as psc,
                tc.tile_pool(name="pout", bufs=2, space="PSUM") as pout,
            ):
                # ================= prologue =================
                nc.sync.dma_start(out=ident[:], in_=ident_in[:])
                nc.sync.dma_start(out=ones_r[:], in_=ones_in[:])
                nc.sync.dma_start(out=tri[:], in_=tri_in[:])
                nc.vector.memset(eps_t[:], 1e-5)
                nc.vector.memset(vones[:], 1.0)

                if not zero_lmb:
                    def bcast_row(dst_ap, src_dram_ap, n):
                        done = 0
                        while done < n:
                            w = min(512, n - done)
                            nc.sync.dma_start(out=brow[:, :w], in_=src_dram_ap[:, done:done + w])
                            nc.vector.tensor_copy(out=brow_bf[:, :w], in_=brow[:, :w])
                            ps = pmm.tile([128, 512], F32, tag="pmm")
                            nc.tensor.matmul(ps[:, :w], lhsT=ones_r[:, :], rhs=brow_bf[:, :w],
                                             start=True, stop=True)
                            nc.vector.tensor_copy(out=dst_ap[:, done:done + w], in_=ps[:, :w])
                            done += w
                    bcast_row(lmb_bc[:, :], lmb[:, :], VSH)

                # embeddings -> residual h [128 tok, 4 blocks * 512 feat] fp32
                for t in range(4):
                    nc.sync.dma_start(out=idx_sb[:, t:t + 1], in_=xidx[128 * t:128 * (t + 1), :])
                nc.sync.dma_start(out=lmidx_sb[:], in_=lmidx[:])
                for t in range(4):
                    emb = wk.tile([128, D], F32, tag="emb", bufs=1)
                    nc.gpsimd.indirect_dma_start(
                        out=emb[:], out_offset=None, in_=tok_emb[:],
                        in_offset=bass.IndirectOffsetOnAxis(ap=idx_sb[:, t:t + 1], axis=0),
                    )
                    pos_t = wk.tile([128, D], F32, tag="emb2", bufs=1)
                    nc.sync.dma_start(out=pos_t[:], in_=pos[128 * t:128 * (t + 1), :])
                    nc.vector.tensor_tensor(out=hts[t][:], in0=emb[:], in1=pos_t[:], op=OP.add)

                # ---- helpers ----
                def layernorm_T(srcs, dst_bf_T):
                    """srcs: 4 tiles [128, D] fp32 [tok, feat]. Writes dst_bf_T
                    [128, 4*512] bf16 = transposed ([feat-ptile, tok]) normalized."""
                    hln = wk.tile([128, 4 * D], BF16, tag="hln", bufs=1)
                    nmu4 = wk.tile([128, 4], F32, tag="lnmu")
                    s4 = wk.tile([128, 4], F32, tag="lns4")
                    ssq4 = wk.tile([128, 4], F32, tag="lnssq")
                    mu2 = wk.tile([128, 4], F32, tag="lnmu2")
                    var4 = wk.tile([128, 4], F32, tag="lnvar")
                    rs4 = wk.tile([128, 4], F32, tag="lnrs")
                    for t in range(4):
                        sq = wk.tile([128, D], F32, tag="lnsq", bufs=2)
                        nc.vector.tensor_reduce(out=s4[:, t:t + 1], in_=srcs[t][:],
                                                axis=mybir.AxisListType.X, op=OP.add)
                        nc.vector.tensor_tensor(out=sq[:], in0=srcs[t][:], in1=srcs[t][:],
                                                op=OP.mult)
                        nc.vector.tensor_reduce(out=ssq4[:, t:t + 1], in_=sq[:],
                                                axis=mybir.AxisListType.X, op=OP.add)
                    nc.vector.tensor_scalar_mul(nmu4[:], s4[:], -1.0 / D)
                    nc.vector.tensor_tensor(out=mu2[:], in0=nmu4[:], in1=nmu4[:], op=OP.mult)
                    nc.vector.scalar_tensor_tensor(out=var4[:], in0=ssq4[:], scalar=1.0 / D,
                                                   in1=mu2[:], op0=OP.mult, op1=OP.subtract)
                    nc.scalar.activation(out=rs4[:], in_=var4[:], func=AF.Ln, bias=eps_t[:])
                    nc.scalar.activation(out=rs4[:], in_=rs4[:], func=AF.Exp, scale=-0.5)
                    for t in range(4):
                        nc.vector.tensor_scalar(out=hln[:, D * t:D * (t + 1)], in0=srcs[t][:],
                                                scalar1=nmu4[:, t:t + 1], scalar2=rs4[:, t:t + 1],
                                                op0=OP.add, op1=OP.mult)
                    for f in range(4):
                        tp = pmm.tile([128, 512], BF16, tag="pmm")
                        for t in range(4):
                            nc.tensor.transpose(out=tp[:, 128 * t:128 * (t + 1)],
                                                in_=hln[:, D * t + 128 * f: D * t + 128 * (f + 1)],
                                                identity=ident[:])
                        nc.vector.tensor_copy(out=dst_bf_T[:, 512 * f:512 * (f + 1)], in_=tp[:])

                # ================= transformer layers =================
                for l in range(L):
                    wq = wp.tile([128, 4 * 1536], BF16, tag="wq")
                    nc.sync.dma_start(out=wq[:].rearrange("p (c e) -> p c e", c=4),
                                      in_=wqkvT[l].rearrange("(c p) e -> p c e", p=128))
                    wpj = wp.tile([128, 4 * 512], BF16, tag="wpj")
                    nc.sync.dma_start(out=wpj[:].rearrange("p (c e) -> p c e", c=4),
                                      in_=wprojT[l].rearrange("(c p) e -> p c e", p=128))
                    wf1 = wp.tile([128, 4 * 2048], BF16, tag="wf1")
                    nc.sync.dma_start(out=wf1[:].rearrange("p (c e) -> p c e", c=4),
                                      in_=wffn1T[l].rearrange("(c p) e -> p c e", p=128))
                    wf2 = wp.tile([128, 16 * 512], BF16, tag="wf2")
                    nc.sync.dma_start(out=wf2[:].rearrange("p (c e) -> p c e", c=16),
                                      in_=wffn2T[l].rearrange("(c p) e -> p c e", p=128))
                    bq = wp.tile([128, 12], F32, tag="bq")
                    nc.sync.dma_start(out=bq[:],
                                      in_=bqkv[l * 1536:(l + 1) * 1536, :].rearrange(
                                          "(o p) x -> p (o x)", p=128))
                    bf1 = wp.tile([128, 16], F32, tag="bf1")
                    nc.sync.dma_start(out=bf1[:],
                                      in_=bffn1[l * 2048:(l + 1) * 2048, :].rearrange(
                                          "(o p) x -> p (o x)", p=128))
                    bprow = wp.tile([1, 512], BF16, tag="bprow")
                    nc.sync.dma_start(out=bprow[:], in_=bproj[l])
                    bf2row = wp.tile([1, 512], BF16, tag="bf2row")
                    nc.sync.dma_start(out=bf2row[:], in_=bffn2[l])

                    # -- LN1 + transpose --
                    hlnT = wk.tile([128, 4 * 512], BF16, tag="hlnT", bufs=1)
                    layernorm_T(hts, hlnT)

                    # -- qkvT = W' @ hlnT ([3D feat, 512 tok]); k,v first, bounce
                    # DMAs issued per-block so the kv AllToAll fires early --
                    qkvT = wk.tile([128, 12 * 512], BF16, tag="qkvT", bufs=1)
                    for o in [4, 5, 6, 7, 8, 9, 10, 11, 0, 1, 2, 3]:
                        ps = pmm.tile([128, 512], F32, tag="pmm")
                        for kc in range(4):
                            nc.tensor.matmul(ps[:],
                                             lhsT=wq[:, 1536 * kc + 128 * o:1536 * kc + 128 * (o + 1)],
                                             rhs=hlnT[:, 512 * kc:512 * (kc + 1)],
                                             start=(kc == 0), stop=(kc == 3))
                        nc.scalar.activation(out=qkvT[:, 512 * o:512 * (o + 1)], in_=ps[:],
                                             func=AF.Identity, bias=bq[:, o:o + 1])
                        if 4 <= o < 8:        # k block: shards s = 2(o-4)+hh
                            for hh in (0, 1):
                                s_ = 2 * (o - 4) + hh
                                nc.sync.dma_start(out=kv_ai[l][128 * s_:128 * s_ + 64, :],
                                                  in_=qkvT[64 * hh:64 * hh + 64, 512 * o:512 * (o + 1)])
                        elif o >= 8:          # v block
                            for hh in (0, 1):
                                s_ = 2 * (o - 8) + hh
                                nc.sync.dma_start(out=kv_ai[l][128 * s_ + 64:128 * s_ + 128, :],
                                                  in_=qkvT[64 * hh:64 * hh + 64, 512 * o:512 * (o + 1)])
                            if o == 11:
                                nc.gpsimd.collective_compute(
                                    "AllToAll", OP.bypass, replica_groups=grp,
                                    ins=[kv_ai[l][:]], outs=[kv_ao[l][:]],
                                )
                        else:                 # q block
                            for hh in (0, 1):
                                s_ = 2 * o + hh
                                nc.sync.dma_start(out=q_ai[l][64 * s_:64 * (s_ + 1), :],
                                                  in_=qkvT[64 * hh:64 * hh + 64, 512 * o:512 * (o + 1)])
                            if o == 3:
                                nc.gpsimd.collective_compute(
                                    "AllToAll", OP.bypass, replica_groups=grp,
                                    ins=[q_ai[l][:]], outs=[q_ao[l][:]],
                                )
                    # assemble k/v first; v transposes run during the q AllToAll
                    qT = wk.tile([128, 2048], BF16, tag="qT", bufs=1)
                    kT = wk.tile([128, 2048], BF16, tag="kT", bufs=1)
                    vT = wk.tile([128, 2048], BF16, tag="vT", bufs=1)
                    for r in range(8):
                        b_, rr = r // 4, r % 4
                        nc.sync.dma_start(out=kT[64 * b_:64 * b_ + 64, 512 * rr:512 * (rr + 1)],
                                          in_=kv_ao[l][128 * r:128 * r + 64, :])
                        nc.sync.dma_start(out=vT[64 * b_:64 * b_ + 64, 512 * rr:512 * (rr + 1)],
                                          in_=kv_ao[l][128 * r + 64:128 * r + 128, :])
                    for b_ in range(2):
                        hb = 64 * b_
                        for i in range(16):
                            tp = pmm.tile([128, 64], BF16, tag="pmm")
                            nc.tensor.transpose(out=tp[:], in_=vT[hb:hb + 64, 128 * i:128 * (i + 1)],
                                                identity=ident[hb:hb + 64, hb:hb + 64])
                            nc.vector.tensor_copy(out=vones[:, 65 * (16 * b_ + i):65 * (16 * b_ + i) + 64],
                                                  in_=tp[:])
                    for r in range(8):
                        b_, rr = r // 4, r % 4
                        nc.sync.dma_start(out=qT[64 * b_:64 * b_ + 64, 512 * rr:512 * (rr + 1)],
                                          in_=q_ao[l][64 * r:64 * (r + 1), :])

                    # -- attention (my head, both batches, causal); score of item
                    # t+1 issues before AV of item t so PE never waits on exp --
                    attnT = wk.tile([64, 4096], BF16, tag="attnT", bufs=1)
                    for b_ in range(2):
                        hb = 64 * b_
                        for p in range(2):
                            outp = pout.tile([65, 1024], F32, tag="pout")
                            items = []
                            for i in range(8 * p + 8):
                                jlmin = max(0, i // 4 - 2 * p)
                                for jl in (0, 1):
                                    if jl < jlmin:
                                        continue
                                    diag = (i // 4 == 2 * p + jl)
                                    c0 = 128 * (i % 4) if diag else 0
                                    items.append((i, jl, diag, c0))

                            def score(it):
                                i, jl, diag, c0 = it
                                sc = psc.tile([128, 512], F32, tag="psc")
                                nc.tensor.matmul(
                                    sc[:, c0:512],
                                    lhsT=kT[hb:hb + 64, 128 * i:128 * (i + 1)],
                                    rhs=qT[hb:hb + 64,
                                           1024 * p + 512 * jl + c0:1024 * p + 512 * (jl + 1)],
                                    start=True, stop=True)
                                ex = ep.tile([128, 512], BF16, tag="ex")
                                nc.scalar.activation(out=ex[:, c0:512], in_=sc[:, c0:512],
                                                     func=AF.Exp, scale=float(HD) ** -0.5)
                                if diag:
                                    nc.vector.tensor_tensor(out=ex[:, c0:c0 + 128],
                                                            in0=ex[:, c0:c0 + 128],
                                                            in1=tri[:], op=OP.mult)
                                return ex

                            def av(it, ex):
                                i, jl, diag, c0 = it
                                kmax = 4 * (2 * p + jl) + 3
                                nc.tensor.matmul(
                                    outp[:, 512 * jl + c0:512 * (jl + 1)],
                                    lhsT=vones[:, 65 * (16 * b_ + i):65 * (16 * b_ + i + 1)],
                                    rhs=ex[:, c0:512],
                                    start=(i == 0), stop=(i == kmax))

                            pend = None
                            for it in items:
                                ex = score(it)
                                if pend is not None:
                                    av(*pend)
                                pend = (it, ex)
                            av(*pend)

                            # normalize: rows 0..63 /= row 64
                            dnb = wk.tile([1, 1024], BF16, tag="rcb", bufs=1)
                            nc.vector.tensor_copy(out=dnb[:], in_=outp[64:65, :])
                            for q2 in range(2):
                                bc = psc.tile([64, 512], F32, tag="psc")
                                nc.tensor.matmul(bc[:], lhsT=ones_r[:, 0:64],
                                                 rhs=dnb[:, 512 * q2:512 * (q2 + 1)], start=True, stop=True)
                                rcs = wk.tile([64, 512], F32, tag="bcs", bufs=1)
                                nc.vector.reciprocal_approx_fast(out=rcs[:], in_=bc[:])
                                nc.vector.tensor_tensor(
                                    out=attnT[:, 2048 * b_ + 1024 * p + 512 * q2:2048 * b_ + 1024 * p + 512 * (q2 + 1)],
                                    in0=outp[0:64, 512 * q2:512 * (q2 + 1)], in1=rcs[:], op=OP.mult)
                    # A2A attention outputs back to token owners
                    for s_ in range(8):
                        nc.sync.dma_start(out=att_ai[l][64 * s_:64 * (s_ + 1), :],
                                          in_=attnT[:, 512 * s_:512 * (s_ + 1)])
                    nc.gpsimd.collective_compute(
                        "AllToAll", OP.bypass, replica_groups=grp,
                        ins=[att_ai[l][:]], outs=[att_ao[l][:]],
                    )
                    aT = wk.tile([128, 4 * 512], BF16, tag="aT", bufs=1)
                    for r in range(4):
                        nc.sync.dma_start(out=aT[:, 512 * r:512 * (r + 1)],
                                          in_=att_ao[l][128 * r:128 * (r + 1), :])

                    # -- proj + residual (bias via rank-1 matmul) --
                    for t in range(4):
                        ps = pmm.tile([128, 512], F32, tag="pmm")
                        for fc in range(4):
                            nc.tensor.matmul(ps[:],
                                             lhsT=aT[:, 512 * fc + 128 * t:512 * fc + 128 * (t + 1)],
                                             rhs=wpj[:, 512 * fc:512 * (fc + 1)],
                                             start=(fc == 0), stop=False)
                        nc.tensor.matmul(ps[:], lhsT=ones_r[:, 0:128], rhs=bprow[:],
                                         start=False, stop=True)
                        nc.vector.tensor_tensor(out=hts[t][:], in0=hts[t][:], in1=ps[:], op=OP.add)

                    # -- LN2 + FFN --
                    hln2T = wk.tile([128, 4 * 512], BF16, tag="hlnT", bufs=1)
                    layernorm_T(hts, hln2T)
                    fT = wk.tile([128, 16 * 512], BF16, tag="fT", bufs=1)
                    for o in range(16):
                        ps = pmm.tile([128, 512], F32, tag="pmm")
                        for kc in range(4):
                            nc.tensor.matmul(ps[:],
                                             lhsT=wf1[:, 2048 * kc + 128 * o:2048 * kc + 128 * (o + 1)],
                                             rhs=hln2T[:, 512 * kc:512 * (kc + 1)],
                                             start=(kc == 0), stop=(kc == 3))
                        nc.scalar.activation(out=fT[:, 512 * o:512 * (o + 1)], in_=ps[:],
                                             func=AF.Gelu, bias=bf1[:, o:o + 1])
                    for t in range(4):
                        ps = pmm.tile([128, 512], F32, tag="pmm")
                        for kc in range(16):
                            nc.tensor.matmul(ps[:],
                                             lhsT=fT[:, 512 * kc + 128 * t:512 * kc + 128 * (t + 1)],
                                             rhs=wf2[:, 512 * kc:512 * (kc + 1)],
                                             start=(kc == 0), stop=False)
                        nc.tensor.matmul(ps[:], lhsT=ones_r[:, 0:128], rhs=bf2row[:],
                                         start=False, stop=True)
                        nc.vector.tensor_tensor(out=hts[t][:], in0=hts[t][:], in1=ps[:], op=OP.add)

                # ================= final LN + AllGather =================
                layernorm_T(hts, hfT)
                for f in range(4):
                    nc.sync.dma_start(out=ag_in[128 * f:128 * (f + 1), :],
                                      in_=hfT[:, 512 * f:512 * (f + 1)])
                nc.gpsimd.collective_compute(
                    "AllGather", OP.bypass, replica_groups=[list(range(NC))],
                    ins=[ag_in[:]], outs=[ag_out[:]],
                )

            # ================= LM head (body pools closed, PSUM free) ========
            with (
                tc.tile_pool(name="lmw", bufs=1) as lw,
                tc.tile_pool(name="lmwork", bufs=3) as lk,
                tc.tile_pool(name="lmgat", bufs=2) as lg,
                tc.tile_pool(name="plm", bufs=2, space="PSUM") as plm,
            ):
                lmw = lw.tile([128, 4 * VSH], BF16, name="lmw")
                nc.sync.dma_start(out=lmw[:].rearrange("p (c e) -> p c e", c=4),
                                  in_=lmT[:].rearrange("(c p) e -> p c e", p=128))

                def lm_tile(lhs_slices, out_row):
                    stage = lk.tile([128, VSH], F32, tag="stage")
                    for hf in range(2):
                        ps = plm.tile([128, 2048], F32, tag="plm")
                        for kc in range(4):
                            for vc in range(4):
                                w0 = VSH * kc + 2000 * hf + 500 * vc
                                nc.tensor.matmul(
                                    ps[:, 512 * vc:512 * vc + 500],
                                    lhsT=lhs_slices[kc],
                                    rhs=lmw[:, w0:w0 + 500],
                                    start=(kc == 0), stop=(kc == 3))
                        ps3 = ps[:].rearrange("p (b e) -> p b e", b=4)
                        st3 = stage[:, 2000 * hf:2000 * (hf + 1)].rearrange("p (b e) -> p b e", b=4)
                        if zero_lmb:
                            nc.vector.tensor_copy(out=st3[:, 0:2, :], in_=ps3[:, 0:2, 0:500])
                            nc.scalar.copy(out=st3[:, 2:4, :], in_=ps3[:, 2:4, 0:500])
                        else:
                            bc3 = lmb_bc[:, 2000 * hf:2000 * (hf + 1)].rearrange("p (b e) -> p b e", b=4)
                            nc.vector.tensor_tensor(out=st3, in0=ps3[:, :, 0:500], in1=bc3, op=OP.add)
                    nc.sync.dma_start(out=logits[out_row:out_row + 128, :], in_=stage[:])

                # 4 local token tiles first (read hfT directly; overlaps the AllGather)
                for u in range(4):
                    lm_tile([hfT[:, 512 * kc + 128 * u:512 * kc + 128 * (u + 1)] for kc in range(4)],
                            128 * u)
                # 7 remote ranks (rotated order (c+1+k)%8); one gather per (rank, kc)
                for k in range(7):
                    rksb = lg.tile([128, 4 * 512], BF16, tag="rk")
                    for kc in range(4):
                        nc.gpsimd.indirect_dma_start(
                            out=rksb[:, 512 * kc:512 * (kc + 1)], out_offset=None,
                            in_=ag_out[:],
                            in_offset=bass.IndirectOffsetOnAxis(
                                ap=lmidx_sb[:, 4 * k + kc:4 * k + kc + 1], axis=0),
                        )
                    for u in range(4):
                        lm_tile([rksb[:, 512 * kc + 128 * u:512 * kc + 128 * (u + 1)]
                                 for kc in range(4)],
                                512 * (k + 1) + 128 * u)

    nc.compile()
    return nc


_NC_CACHE = {}


def _get_nc(zero_lmb: bool = True):
    if zero_lmb not in _NC_CACHE:
        _NC_CACHE[zero_lmb] = build_nc(zero_lmb)
    return _NC_CACHE[zero_lmb]


def _prep_inputs(inputs):
    bf = ml_dtypes.bfloat16
    tok_emb = np.asarray(inputs["tok_emb"], np.float32)
    pos_emb = np.asarray(inputs["pos_emb"], np.float32)
    x = np.asarray(inputs["x"]).astype(np.int32).reshape(-1)  # [4096] flat

    def eff(w, g, b, wb):
        # fold the preceding layernorm's gamma/beta into w (out,in) and bias
        w = np.asarray(w, np.float32)
        weff = w * np.asarray(g, np.float32)[None, :]
        beff = w @ np.asarray(b, np.float32) + np.asarray(wb, np.float32)
        return weff, beff

    wqkvT = np.zeros((L, D, 3 * D), bf)
    bqkv = np.zeros((L, 12, 128), np.float32)
    wprojT = np.zeros((L, D, D), bf)
    bproj = np.zeros((L, 1, D), bf)
    wffn1T = np.zeros((L, D, DFF), bf)
    bffn1 = np.zeros((L, 16, 128), np.float32)
    wffn2T = np.zeros((L, DFF, D), bf)
    bffn2 = np.zeros((L, 1, D), bf)
    for l in range(L):
        w, b = eff(inputs["qkv_w"][l], inputs["ln1_g"][l], inputs["ln1_b"][l], inputs["qkv_b"][l])
        wqkvT[l] = w.T.astype(bf)
        bqkv[l] = b.reshape(12, 128)
        wprojT[l] = np.asarray(inputs["proj_w"][l], np.float32).T.astype(bf)
        bproj[l, 0] = np.asarray(inputs["proj_b"][l], np.float32).astype(bf)
        w, b = eff(inputs["ffn1_w"][l], inputs["ln2_g"][l], inputs["ln2_b"][l], inputs["ffn1_b"][l])
        wffn1T[l] = w.T.astype(bf)
        bffn1[l] = b.reshape(16, 128)
        wffn2T[l] = np.asarray(inputs["ffn2_w"][l], np.float32).T.astype(bf)
        bffn2[l, 0] = np.asarray(inputs["ffn2_b"][l], np.float32).astype(bf)
    lmw, lmbf = eff(inputs["lm_w"], inputs["lnf_g"], inputs["lnf_b"], inputs["lm_b"])
    zero_lmb = not np.any(lmbf)

    tri_m = (np.arange(128)[:, None] <= np.arange(128)[None, :]).astype(bf)

    common = dict(tok_emb=tok_emb, wqkvT=wqkvT, bqkv=bqkv.reshape(L * 12 * 128, 1),
                  wprojT=wprojT, bproj=bproj, wffn1T=wffn1T,
                  bffn1=bffn1.reshape(L * 16 * 128, 1), wffn2T=wffn2T, bffn2=bffn2,
                  tri_in=tri_m, ident_in=np.eye(128, dtype=bf),
                  ones_in=np.ones((1, 128), bf))
    in_maps = []
    pvec = np.arange(128, dtype=np.int32)
    for c in range(NC):
        s0 = 512 * (c % 4)
        m = dict(common)
        m["pos"] = pos_emb[s0:s0 + 512]
        m["xidx"] = x[512 * c:512 * (c + 1)].reshape(TOK, 1)
        lmidx = np.zeros((128, 28), np.int32)
        for k in range(7):
            rk = (c + 1 + k) % NC
            for kc in range(4):
                lmidx[:, 4 * k + kc] = 512 * rk + 128 * kc + pvec  # gather rows
        m["lmidx"] = lmidx
        m["lmT"] = np.ascontiguousarray(lmw[VSH * c:VSH * (c + 1)].T.astype(bf))
        m["lmb"] = lmbf[VSH * c:VSH * (c + 1)].reshape(1, VSH).copy()
        in_maps.append(m)
    return in_maps, zero_lmb


def run(inputs, trace=False, tmpdir=None):
    in_maps, zero_lmb = _prep_inputs(inputs)
    nc = _get_nc(zero_lmb)
    res = bass_utils.run_bass_kernel_spmd(nc, in_maps, list(range(NC)), trace=trace, tmpdir=tmpdir)
    full = np.empty((B * S, V), np.float32)
    for c in range(NC):
        # core c writes its logits rows rotated by -512c; un-rotate
        full[:, VSH * c:VSH * (c + 1)] = np.roll(res.results[c]["logits"], 512 * c, axis=0)
    return full.reshape(B, S, V), res


def kernel(**inputs) -> np.ndarray:
    out, _ = run(inputs)
    return out


# revision 27
# speedup vs baseline: 1.1622x; 1.0104x over previous
"""MiniGPT forward pass on 8 Trainium2 NeuronCores (Bass/Tile SPMD kernel).

Model: V=32000, T=2048, D=512, H=8 heads, L=4 layers, DFF=2048, B=2, S=2048.

Sharding (8 cores, one SPMD program):
- Tokens: core c owns 512 tokens = flat[512c : 512c+512] (batch c//4).
- Attention: head-parallel; core c computes head c for both batches (batch b
  in partition half 64b) over the batch's full 2048 tokens. QKV and attention
  outputs are redistributed with AllToAll over all 8 cores.
- LM head: vocab-parallel; core c computes logits[:, 4000c:4000c+4000] for all
  4096 tokens after an AllGather of the final hidden states. Logit rows are
  written rotated by -512c (so row offsets are core-independent); the host
  un-rotates with np.roll.

Layouts: residual h is [token, feature] fp32 in SBUF. LN outputs are cast to
bf16 and PE-transposed to [feature, token] as matmul operands. LN gamma/beta
are folded into the following matmul weights on the host. proj/ffn2 biases are
added via rank-1 matmuls into PSUM; qkv bias rides the ACT-engine PSUM->SBUF
copy. The causal mask is applied by skipping fully-masked 128-col strips and
multiplying one 128x128 triangular 0/1 mask after exp on the DVE.
"""
import sys

sys.path.insert(0, "/opt/trn_rl_repo")

import numpy as np
import ml_dtypes

import concourse.bass as bass
import concourse.mybir as mybir
import concourse.tile as tile
from concourse import bacc, bass_utils

BF16 = mybir.dt.bfloat16
F32 = mybir.dt.float32
I32 = mybir.dt.int32
AF = mybir.ActivationFunctionType
OP = mybir.AluOpType

V, T, D, H, L = 32000, 2048, 512, 8, 4
HD = D // H          # 64
DFF = 4 * D          # 2048
B, S = 2, 2048
NC = 8               # cores
TOK = 512            # tokens per core
VSH = V // NC        # 4000 vocab per core


def build_nc(zero_lmb: bool):
    nc = bacc.Bacc("TRN2", target_bir_lowering=False, debug=False, num_devices=NC)

    # ---- I/O ----
    tok_emb = nc.dram_tensor("tok_emb", [V, D], F32, kind="ExternalInput")
    pos = nc.dram_tensor("pos", [TOK, D], F32, kind="ExternalInput")
    xidx = nc.dram_tensor("xidx", [TOK, 1], I32, kind="ExternalInput")
    wqkvT = nc.dram_tensor("wqkvT", [L, D, 3 * D], BF16, kind="ExternalInput")
    bqkv = nc.dram_tensor("bqkv", [L * 12 * 128, 1], F32, kind="ExternalInput")
    wprojT = nc.dram_tensor("wprojT", [L, D, D], BF16, kind="ExternalInput")
    bproj = nc.dram_tensor("bproj", [L, 1, D], BF16, kind="ExternalInput")
    wffn1T = nc.dram_tensor("wffn1T", [L, D, DFF], BF16, kind="ExternalInput")
    bffn1 = nc.dram_tensor("bffn1", [L * 16 * 128, 1], F32, kind="ExternalInput")
    wffn2T = nc.dram_tensor("wffn2T", [L, DFF, D], BF16, kind="ExternalInput")
    bffn2 = nc.dram_tensor("bffn2", [L, 1, D], BF16, kind="ExternalInput")
    lmT = nc.dram_tensor("lmT", [D, VSH], BF16, kind="ExternalInput")
    lmb = nc.dram_tensor("lmb", [1, VSH], F32, kind="ExternalInput")
    ident_in = nc.dram_tensor("ident_in", [128, 128], BF16, kind="ExternalInput")
    ones_in = nc.dram_tensor("ones_in", [1, 128], BF16, kind="ExternalInput")
    tri_in = nc.dram_tensor("tri_in", [128, 128], BF16, kind="ExternalInput")
    logits = nc.dram_tensor("logits", [B * S, VSH], F32, kind="ExternalOutput")

    lmidx = nc.dram_tensor("lmidx", [128, 28], I32, kind="ExternalInput")

    # ---- internal DRAM (collective bounces) ----
    # per-shard rows: 64 k + 64 v + 64 q = 192 (q rides the kv AllToAll)
    kv_ai = [nc.dram_tensor(f"kv_ai{l}", [3 * D, TOK], BF16) for l in range(L)]
    kv_ao = [nc.dram_tensor(f"kv_ao{l}", [3 * D, TOK], BF16) for l in range(L)]
    att_ai = [nc.dram_tensor(f"att_ai{l}", [D, TOK], BF16) for l in range(L)]
    att_ao = [nc.dram_tensor(f"att_ao{l}", [D, TOK], BF16) for l in range(L)]
    ag_in = nc.dram_tensor("ag_in", [D, TOK], BF16)
    ag_out = nc.dram_tensor("ag_out", [NC * D, TOK], BF16, addr_space="Shared")
    grp = [list(range(NC))]

    with tile.TileContext(nc) as tc:
        with (
            tc.tile_pool(name="const", bufs=1) as cp,
            tc.tile_pool(name="persist", bufs=1) as pp,
        ):
            ident = cp.tile([128, 128], BF16, name="ident")
            ones_r = cp.tile([1, 128], BF16, name="ones_r")
            tri = cp.tile([128, 128], BF16, name="tri")
            eps_t = cp.tile([128, 1], F32, name="eps_t")
            if not zero_lmb:
                lmb_bc = cp.tile([128, VSH], BF16, name="lmb_bc")
                brow = cp.tile([1, 512], F32, name="brow")
                brow_bf = cp.tile([1, 512], BF16, name="brow_bf")
            hts = [pp.tile([128, D], F32, name=f"h{t}") for t in range(4)]
            idx_sb = pp.tile([128, 4], I32, name="idx_sb")
            lmidx_sb = pp.tile([128, 28], I32, name="lmidx_sb")
            vones = pp.tile([128, 32 * 65], BF16, name="vones")
            hfT = pp.tile([128, 4 * 512], BF16, name="hfT")

            with (
                tc.tile_pool(name="wpool", bufs=2) as wp,
                tc.tile_pool(name="work", bufs=2) as wk,
                tc.tile_pool(name="exppool", bufs=3) as ep,
                tc.tile_pool(name="pmm", bufs=2, space="PSUM") as pmm,
                tc.tile_pool(name="psc", bufs=2, space="PSUM") as psc,
                tc.tile_pool(name="pout", bufs=2, space="PSUM") as pout,
            ):
                # ================= prologue =================
                nc.sync.dma_start(out=ident[:], in_=ident_in[:])
                nc.sync.dma_start(out=ones_r[:], in_=ones_in[:])
                nc.sync.dma_start(out=tri[:], in_=tri_in[:])
                nc.vector.memset(eps_t[:], 1e-5)
                nc.vector.memset(vones[:], 1.0)

                if not zero_lmb:
                    def bcast_row(dst_ap, src_dram_ap, n):
                        done = 0
                        while done < n:
                            w = min(512, n - done)
                            nc.sync.dma_start(out=brow[:, :w], in_=src_dram_ap[:, done:done + w])
                            nc.vector.tensor_copy(out=brow_bf[:, :w], in_=brow[:, :w])
                            ps = pmm.tile([128, 512], F32, tag="pmm")
                            nc.tensor.matmul(ps[:, :w], lhsT=ones_r[:, :], rhs=brow_bf[:, :w],
                                             start=True, stop=True)
                            nc.vector.tensor_copy(out=dst_ap[:, done:done + w], in_=ps[:, :w])
                            done += w
                    bcast_row(lmb_bc[:, :], lmb[:, :], VSH)

                # embeddings -> residual h [128 tok, 4 blocks * 512 feat] fp32
                for t in range(4):
                    nc.sync.dma_start(out=idx_sb[:, t:t + 1], in_=xidx[128 * t:128 * (t + 1), :])
                nc.sync.dma_start(out=lmidx_sb[:], in_=lmidx[:])
                for t in range(4):
                    emb = wk.tile([128, D], F32, tag="emb", bufs=1)
                    nc.gpsimd.indirect_dma_start(
                        out=emb[:], out_offset=None, in_=tok_emb[:],
                        in_offset=bass.IndirectOffsetOnAxis(ap=idx_sb[:, t:t + 1], axis=0),
                    )
                    pos_t = wk.tile([128, D], F32, tag="emb2", bufs=1)
                    nc.sync.dma_start(out=pos_t[:], in_=pos[128 * t:128 * (t + 1), :])
                    nc.vector.tensor_tensor(out=hts[t][:], in0=emb[:], in1=pos_t[:], op=OP.add)

                # ---- helpers ----
                def layernorm_T(srcs, dst_bf_T):
                    """srcs: 4 tiles [128, D] fp32 [tok, feat]. Writes dst_bf_T
                    [128, 4*512] bf16 = transposed ([feat-ptile, tok]) normalized."""
                    hln = wk.tile([128, 4 * D], BF16, tag="hln", bufs=1)
                    nmu4 = wk.tile([128, 4], F32, tag="lnmu")
                    s4 = wk.tile([128, 4], F32, tag="lns4")
                    ssq4 = wk.tile([128, 4], F32, tag="lnssq")
                    mu2 = wk.tile([128, 4], F32, tag="lnmu2")
                    var4 = wk.tile([128, 4], F32, tag="lnvar")
                    rs4 = wk.tile([128, 4], F32, tag="lnrs")
                    for t in range(4):
                        sq = wk.tile([128, D], F32, tag="lnsq", bufs=2)
                        nc.vector.tensor_reduce(out=s4[:, t:t + 1], in_=srcs[t][:],
                                                axis=mybir.AxisListType.X, op=OP.add)
                        nc.scalar.activation(out=sq[:], in_=srcs[t][:], func=AF.Square,
                                             accum_out=ssq4[:, t:t + 1])
                    nc.vector.tensor_scalar_mul(nmu4[:], s4[:], -1.0 / D)
                    nc.vector.tensor_tensor(out=mu2[:], in0=nmu4[:], in1=nmu4[:], op=OP.mult)
                    nc.vector.scalar_tensor_tensor(out=var4[:], in0=ssq4[:], scalar=1.0 / D,
                                                   in1=mu2[:], op0=OP.mult, op1=OP.subtract)
                    nc.scalar.activation(out=rs4[:], in_=var4[:], func=AF.Ln, bias=eps_t[:])
                    nc.scalar.activation(out=rs4[:], in_=rs4[:], func=AF.Exp, scale=-0.5)
                    for t in range(4):
                        nc.vector.tensor_scalar(out=hln[:, D * t:D * (t + 1)], in0=srcs[t][:],
                                                scalar1=nmu4[:, t:t + 1], scalar2=rs4[:, t:t + 1],
                                                op0=OP.add, op1=OP.mult)
                    for f in range(4):
                        tp = pmm.tile([128, 512], BF16, tag="pmm")
                        for t in range(4):
                            nc.tensor.transpose(out=tp[:, 128 * t:128 * (t + 1)],
                                                in_=hln[:, D * t + 128 * f: D * t + 128 * (f + 1)],
                                                identity=ident[:])
                        nc.vector.tensor_copy(out=dst_bf_T[:, 512 * f:512 * (f + 1)], in_=tp[:])

                # ================= transformer layers =================
                for l in range(L):
                    wq = wp.tile([128, 4 * 1536], BF16, tag="wq")
                    nc.sync.dma_start(out=wq[:].rearrange("p (c e) -> p c e", c=4),
                                      in_=wqkvT[l].rearrange("(c p) e -> p c e", p=128))
                    wpj = wp.tile([128, 4 * 512], BF16, tag="wpj")
                    nc.sync.dma_start(out=wpj[:].rearrange("p (c e) -> p c e", c=4),
                                      in_=wprojT[l].rearrange("(c p) e -> p c e", p=128))
                    wf1 = wp.tile([128, 4 * 2048], BF16, tag="wf1")
                    nc.sync.dma_start(out=wf1[:].rearrange("p (c e) -> p c e", c=4),
                                      in_=wffn1T[l].rearrange("(c p) e -> p c e", p=128))
                    wf2 = wp.tile([128, 16 * 512], BF16, tag="wf2")
                    nc.sync.dma_start(out=wf2[:].rearrange("p (c e) -> p c e", c=16),
                                      in_=wffn2T[l].rearrange("(c p) e -> p c e", p=128))
                    bq = wp.tile([128, 12], F32, tag="bq")
                    nc.sync.dma_start(out=bq[:],
                                      in_=bqkv[l * 1536:(l + 1) * 1536, :].rearrange(
                                          "(o p) x -> p (o x)", p=128))
                    bf1 = wp.tile([128, 16], F32, tag="bf1")
                    nc.sync.dma_start(out=bf1[:],
                                      in_=bffn1[l * 2048:(l + 1) * 2048, :].rearrange(
                                          "(o p) x -> p (o x)", p=128))
                    bprow = wp.tile([1, 512], BF16, tag="bprow")
                    nc.sync.dma_start(out=bprow[:], in_=bproj[l])
                    bf2row = wp.tile([1, 512], BF16, tag="bf2row")
                    nc.sync.dma_start(out=bf2row[:], in_=bffn2[l])

                    # -- LN1 + transpose --
                    hlnT = wk.tile([128, 4 * 512], BF16, tag="hlnT", bufs=1)
                    layernorm_T(hts, hlnT)

                    # -- qkvT = W' @ hlnT ([3D feat, 512 tok]); k,v first, bounce
                    # DMAs issued per-block so the kv AllToAll fires early --
                    qkvT = wk.tile([128, 12 * 512], BF16, tag="qkvT", bufs=1)
                    for o in [4, 5, 6, 7, 8, 9, 10, 11, 0, 1, 2, 3]:
                        ps = pmm.tile([128, 512], F32, tag="pmm")
                        for kc in range(4):
                            nc.tensor.matmul(ps[:],
                                             lhsT=wq[:, 1536 * kc + 128 * o:1536 * kc + 128 * (o + 1)],
                                             rhs=hlnT[:, 512 * kc:512 * (kc + 1)],
                                             start=(kc == 0), stop=(kc == 3))
                        nc.scalar.activation(out=qkvT[:, 512 * o:512 * (o + 1)], in_=ps[:],
                                             func=AF.Identity, bias=bq[:, o:o + 1])
                        if 4 <= o < 8:        # k block: shards s = 2(o-4)+hh
                            for hh in (0, 1):
                                s_ = 2 * (o - 4) + hh
                                nc.sync.dma_start(out=kv_ai[l][192 * s_:192 * s_ + 64, :],
                                                  in_=qkvT[64 * hh:64 * hh + 64, 512 * o:512 * (o + 1)])
                        elif o >= 8:          # v block
                            for hh in (0, 1):
                                s_ = 2 * (o - 8) + hh
                                nc.sync.dma_start(out=kv_ai[l][192 * s_ + 64:192 * s_ + 128, :],
                                                  in_=qkvT[64 * hh:64 * hh + 64, 512 * o:512 * (o + 1)])
                        else:                 # q block
                            for hh in (0, 1):
                                s_ = 2 * o + hh
                                nc.sync.dma_start(out=kv_ai[l][192 * s_ + 128:192 * s_ + 192, :],
                                                  in_=qkvT[64 * hh:64 * hh + 64, 512 * o:512 * (o + 1)])
                            if o == 3:
                                nc.gpsimd.collective_compute(
                                    "AllToAll", OP.bypass, replica_groups=grp,
                                    ins=[kv_ai[l][:]], outs=[kv_ao[l][:]],
                                )
                    qT = wk.tile([128, 2048], BF16, tag="qT", bufs=1)
                    kT = wk.tile([128, 2048], BF16, tag="kT", bufs=1)
                    vT = wk.tile([128, 2048], BF16, tag="vT", bufs=1)
                    for r in range(8):
                        b_, rr = r // 4, r % 4
                        nc.sync.dma_start(out=kT[64 * b_:64 * b_ + 64, 512 * rr:512 * (rr + 1)],
                                          in_=kv_ao[l][192 * r:192 * r + 64, :])
                        nc.sync.dma_start(out=vT[64 * b_:64 * b_ + 64, 512 * rr:512 * (rr + 1)],
                                          in_=kv_ao[l][192 * r + 64:192 * r + 128, :])
                        nc.sync.dma_start(out=qT[64 * b_:64 * b_ + 64, 512 * rr:512 * (rr + 1)],
                                          in_=kv_ao[l][192 * r + 128:192 * r + 192, :])
                    for h2 in range(2):
                        hb = 64 * h2
                        for i in range(16):
                            tp = pmm.tile([128, 64], BF16, tag="pmm")
                            nc.tensor.transpose(out=tp[:], in_=vT[hb:hb + 64, 128 * i:128 * (i + 1)],
                                                identity=ident[hb:hb + 64, hb:hb + 64])
                            nc.vector.tensor_copy(out=vones[:, 65 * (16 * h2 + i):65 * (16 * h2 + i) + 64],
                                                  in_=tp[:])

                    # -- attention (my head, both batches, causal); score of item
                    # t+1 issues before AV of item t so PE never waits on exp --
                    attnT = wk.tile([64, 4096], BF16, tag="attnT", bufs=1)
                    for h2 in range(2):
                        hb = 64 * h2
                        for p in range(2):
                            outp = pout.tile([65, 1024], F32, tag="pout")
                            items = []
                            for i in range(8 * p + 8):
                                jlmin = max(0, i // 4 - 2 * p)
                                for jl in (0, 1):
                                    if jl < jlmin:
                                        continue
                                    diag = (i // 4 == 2 * p + jl)
                                    c0 = 128 * (i % 4) if diag else 0
                                    items.append((i, jl, diag, c0))

                            def score(it):
                                i, jl, diag, c0 = it
                                sc = psc.tile([128, 512], F32, tag="psc")
                                nc.tensor.matmul(
                                    sc[:, c0:512],
                                    lhsT=kT[hb:hb + 64, 128 * i:128 * (i + 1)],
                                    rhs=qT[hb:hb + 64,
                                           1024 * p + 512 * jl + c0:1024 * p + 512 * (jl + 1)],
                                    start=True, stop=True)
                                ex = ep.tile([128, 512], BF16, tag="ex")
                                nc.scalar.activation(out=ex[:, c0:512], in_=sc[:, c0:512],
                                                     func=AF.Exp, scale=float(HD) ** -0.5)
                                if diag:
                                    nc.vector.tensor_tensor(out=ex[:, c0:c0 + 128],
                                                            in0=ex[:, c0:c0 + 128],
                                                            in1=tri[:], op=OP.mult)
                                return ex

                            def av(it, ex):
                                i, jl, diag, c0 = it
                                kmax = 4 * (2 * p + jl) + 3
                                nc.tensor.matmul(
                                    outp[:, 512 * jl + c0:512 * (jl + 1)],
                                    lhsT=vones[:, 65 * (16 * h2 + i):65 * (16 * h2 + i + 1)],
                                    rhs=ex[:, c0:512],
                                    start=(i == 0), stop=(i == kmax))

                            pend = None
                            for it in items:
                                ex = score(it)
                                if pend is not None:
                                    av(*pend)
                                pend = (it, ex)
                            av(*pend)

                            # normalize: rows 0..63 /= row 64
                            dnb = wk.tile([1, 1024], BF16, tag="rcb", bufs=1)
                            nc.vector.tensor_copy(out=dnb[:], in_=outp[64:65, :])
                            for q2 in range(2):
                                bc = psc.tile([64, 512], F32, tag="psc")
                                nc.tensor.matmul(bc[:], lhsT=ones_r[:, 0:64],
                                                 rhs=dnb[:, 512 * q2:512 * (q2 + 1)], start=True, stop=True)
                                rcs = wk.tile([64, 512], F32, tag="bcs", bufs=1)
                                nc.vector.reciprocal_approx_fast(out=rcs[:], in_=bc[:])
                                nc.vector.tensor_tensor(
                                    out=attnT[:, 2048 * h2 + 1024 * p + 512 * q2:2048 * h2 + 1024 * p + 512 * (q2 + 1)],
                                    in0=outp[0:64, 512 * q2:512 * (q2 + 1)], in1=rcs[:], op=OP.mult)
                    # A2A attention outputs back to token owners
                    for s_ in range(8):
                        nc.sync.dma_start(out=att_ai[l][64 * s_:64 * (s_ + 1), :],
                                          in_=attnT[:, 512 * s_:512 * (s_ + 1)])
                    nc.gpsimd.collective_compute(
                        "AllToAll", OP.bypass, replica_groups=grp,
                        ins=[att_ai[l][:]], outs=[att_ao[l][:]],
                    )
                    aT = wk.tile([128, 4 * 512], BF16, tag="aT", bufs=1)
                    for r in range(4):
                        nc.sync.dma_start(out=aT[:, 512 * r:512 * (r + 1)],
                                          in_=att_ao[l][128 * r:128 * (r + 1), :])

                    # -- proj + residual (bias via rank-1 matmul) --
                    for t in range(4):
                        ps = pmm.tile([128, 512], F32, tag="pmm")
                        for fc in range(4):
                            nc.tensor.matmul(ps[:],
                                             lhsT=aT[:, 512 * fc + 128 * t:512 * fc + 128 * (t + 1)],
                                             rhs=wpj[:, 512 * fc:512 * (fc + 1)],
                                             start=(fc == 0), stop=False)
                        nc.tensor.matmul(ps[:], lhsT=ones_r[:, 0:128], rhs=bprow[:],
                                         start=False, stop=True)
                        nc.vector.tensor_tensor(out=hts[t][:], in0=hts[t][:], in1=ps[:], op=OP.add)

                    # -- LN2 + FFN; ffn2 tiles t0/t1 accumulate in lockstep with
                    # gelu output so the PE never waits for the gelu tail --
                    hln2T = wk.tile([128, 4 * 512], BF16, tag="hlnT", bufs=1)
                    layernorm_T(hts, hln2T)
                    fT = wk.tile([128, 16 * 512], BF16, tag="fT", bufs=1)
                    psA = psc.tile([128, 512], F32, tag="psc")
                    psB = psc.tile([128, 512], F32, tag="psc")
                    for o in range(16):
                        ps = pmm.tile([128, 512], F32, tag="pmm")
                        for kc in range(4):
                            nc.tensor.matmul(ps[:],
                                             lhsT=wf1[:, 2048 * kc + 128 * o:2048 * kc + 128 * (o + 1)],
                                             rhs=hln2T[:, 512 * kc:512 * (kc + 1)],
                                             start=(kc == 0), stop=(kc == 3))
                        nc.scalar.activation(out=fT[:, 512 * o:512 * (o + 1)], in_=ps[:],
                                             func=AF.Gelu, bias=bf1[:, o:o + 1])
                        for t, pst in ((0, psA), (1, psB)):
                            nc.tensor.matmul(pst[:],
                                             lhsT=fT[:, 512 * o + 128 * t:512 * o + 128 * (t + 1)],
                                             rhs=wf2[:, 512 * o:512 * (o + 1)],
                                             start=(o == 0), stop=False)
                    for t, pst in ((0, psA), (1, psB)):
                        nc.tensor.matmul(pst[:], lhsT=ones_r[:, 0:128], rhs=bf2row[:],
                                         start=False, stop=True)
                        nc.vector.tensor_tensor(out=hts[t][:], in0=hts[t][:], in1=pst[:], op=OP.add)
                    for t in (2, 3):
                        ps = pmm.tile([128, 512], F32, tag="pmm")
                        for kc in range(16):
                            nc.tensor.matmul(ps[:],
                                             lhsT=fT[:, 512 * kc + 128 * t:512 * kc + 128 * (t + 1)],
                                             rhs=wf2[:, 512 * kc:512 * (kc + 1)],
                                             start=(kc == 0), stop=False)
                        nc.tensor.matmul(ps[:], lhsT=ones_r[:, 0:128], rhs=bf2row[:],
                                         start=False, stop=True)
                        nc.vector.tensor_tensor(out=hts[t][:], in0=hts[t][:], in1=ps[:], op=OP.add)

                # ================= final LN + AllGather =================
                layernorm_T(hts, hfT)
                for f in range(4):
                    nc.sync.dma_start(out=ag_in[128 * f:128 * (f + 1), :],
                                      in_=hfT[:, 512 * f:512 * (f + 1)])
                nc.gpsimd.collective_compute(
                    "AllGather", OP.bypass, replica_groups=[list(range(NC))],
                    ins=[ag_in[:]], outs=[ag_out[:]],
                )

            # ================= LM head (body pools closed, PSUM free) ========
            with (
                tc.tile_pool(name="lmw", bufs=1) as lw,
                tc.tile_pool(name="lmwork", bufs=3) as lk,
                tc.tile_pool(name="lmgat", bufs=2) as lg,
                tc.tile_pool(name="plm", bufs=2, space="PSUM") as plm,
            ):
                lmw = lw.tile([128, 4 * VSH], BF16, name="lmw")
                nc.sync.dma_start(out=lmw[:].rearrange("p (c e) -> p c e", c=4),
                                  in_=lmT[:].rearrange("(c p) e -> p c e", p=128))

                def lm_tile(lhs_slices, out_row):
                    stage = lk.tile([128, VSH], F32, tag="stage")
                    for hf in range(2):
                        ps = plm.tile([128, 2048], F32, tag="plm")
                        for kc in range(4):
                            for vc in range(4):
                                w0 = VSH * kc + 2000 * hf + 500 * vc
                                nc.tensor.matmul(
                                    ps[:, 512 * vc:512 * vc + 500],
                                    lhsT=lhs_slices[kc],
                                    rhs=lmw[:, w0:w0 + 500],
                                    start=(kc == 0), stop=(kc == 3))
                        ps3 = ps[:].rearrange("p (b e) -> p b e", b=4)
                        st3 = stage[:, 2000 * hf:2000 * (hf + 1)].rearrange("p (b e) -> p b e", b=4)
                        if zero_lmb:
                            nc.vector.tensor_copy(out=st3[:, 0:2, :], in_=ps3[:, 0:2, 0:500])
                            nc.scalar.copy(out=st3[:, 2:4, :], in_=ps3[:, 2:4, 0:500])
                        else:
                            bc3 = lmb_bc[:, 2000 * hf:2000 * (hf + 1)].rearrange("p (b e) -> p b e", b=4)
                            nc.vector.tensor_tensor(out=st3, in0=ps3[:, :, 0:500], in1=bc3, op=OP.add)
                    nc.sync.dma_start(out=logits[out_row:out_row + 128, :], in_=stage[:])

                # 4 local token tiles first (read hfT directly; overlaps the AllGather)
                for u in range(4):
                    lm_tile([hfT[:, 512 * kc + 128 * u:512 * kc + 128 * (u + 1)] for kc in range(4)],
                            128 * u)
                # 7 remote ranks (rotated order (c+1+k)%8); one gather per (rank, kc)
                for k in range(7):
                    rksb = lg.tile([128, 4 * 512], BF16, tag="rk")
                    for kc in range(4):
                        nc.gpsimd.indirect_dma_start(
                            out=rksb[:, 512 * kc:512 * (kc + 1)], out_offset=None,
                            in_=ag_out[:],
                            in_offset=bass.IndirectOffsetOnAxis(
                                ap=lmidx_sb[:, 4 * k + kc:4 * k + kc + 1], axis=0),
                        )
                    for u in range(4):
                        lm_tile([rksb[:, 512 * kc + 128 * u:512 * kc + 128 * (u + 1)]
                                 for kc in range(4)],
                                512 * (k + 1) + 128 * u)

    nc.compile()
    return nc


_NC_CACHE = {}


def _get_nc(zero_lmb: bool = True):
    if zero_lmb not in _NC_CACHE:
        _NC_CACHE[zero_lmb] = build_nc(zero_lmb)
    return _NC_CACHE[zero_lmb]


def _prep_inputs(inputs):
    bf = ml_dtypes.bfloat16
    tok_emb = np.asarray(inputs["tok_emb"], np.float32)
    pos_emb = np.asarray(inputs["pos_emb"], np.float32)
    x = np.asarray(inputs["x"]).astype(np.int32).reshape(-1)  # [4096] flat

    def eff(w, g, b, wb):
        # fold the preceding layernorm's gamma/beta into w (out,in) and bias
        w = np.asarray(w, np.float32)
        weff = w * np.asarray(g, np.float32)[None, :]
        beff = w @ np.asarray(b, np.float32) + np.asarray(wb, np.float32)
        return weff, beff

    wqkvT = np.zeros((L, D, 3 * D), bf)
    bqkv = np.zeros((L, 12, 128), np.float32)
    wprojT = np.zeros((L, D, D), bf)
    bproj = np.zeros((L, 1, D), bf)
    wffn1T = np.zeros((L, D, DFF), bf)
    bffn1 = np.zeros((L, 16, 128), np.float32)
    wffn2T = np.zeros((L, DFF, D), bf)
    bffn2 = np.zeros((L, 1, D), bf)
    for l in range(L):
        w, b = eff(inputs["qkv_w"][l], inputs["ln1_g"][l], inputs["ln1_b"][l], inputs["qkv_b"][l])
        wqkvT[l] = w.T.astype(bf)
        bqkv[l] = b.reshape(12, 128)
        wprojT[l] = np.asarray(inputs["proj_w"][l], np.float32).T.astype(bf)
        bproj[l, 0] = np.asarray(inputs["proj_b"][l], np.float32).astype(bf)
        w, b = eff(inputs["ffn1_w"][l], inputs["ln2_g"][l], inputs["ln2_b"][l], inputs["ffn1_b"][l])
        wffn1T[l] = w.T.astype(bf)
        bffn1[l] = b.reshape(16, 128)
        wffn2T[l] = np.asarray(inputs["ffn2_w"][l], np.float32).T.astype(bf)
        bffn2[l, 0] = np.asarray(inputs["ffn2_b"][l], np.float32).astype(bf)
    lmw, lmbf = eff(inputs["lm_w"], inputs["lnf_g"], inputs["lnf_b"], inputs["lm_b"])
    zero_lmb = not np.any(lmbf)

    tri_m = (np.arange(128)[:, None] <= np.arange(128)[None, :]).astype(bf)

    common = dict(tok_emb=tok_emb, wqkvT=wqkvT, bqkv=bqkv.reshape(L * 12 * 128, 1),
                  wprojT=wprojT, bproj=bproj, wffn1T=wffn1T,
                  bffn1=bffn1.reshape(L * 16 * 128, 1), wffn2T=wffn2T, bffn2=bffn2,
                  tri_in=tri_m, ident_in=np.eye(128, dtype=bf),
                  ones_in=np.ones((1, 128), bf))
    in_maps = []
    pvec = np.arange(128, dtype=np.int32)
    for c in range(NC):
        s0 = 512 * (c % 4)
        m = dict(common)
        m["pos"] = pos_emb[s0:s0 + 512]
        m["xidx"] = x[512 * c:512 * (c + 1)].reshape(TOK, 1)
        lmidx = np.zeros((128, 28), np.int32)
        for k in range(7):
            rk = (c + 1 + k) % NC
            for kc in range(4):
                lmidx[:, 4 * k + kc] = 512 * rk + 128 * kc + pvec  # gather rows
        m["lmidx"] = lmidx
        m["lmT"] = np.ascontiguousarray(lmw[VSH * c:VSH * (c + 1)].T.astype(bf))
        m["lmb"] = lmbf[VSH * c:VSH * (c + 1)].reshape(1, VSH).copy()
        in_maps.append(m)
    return in_maps, zero_lmb


def run(inputs, trace=False, tmpdir=None):
    in_maps, zero_lmb = _prep_inputs(inputs)
    nc = _get_nc(zero_lmb)
    res = bass_utils.run_bass_kernel_spmd(nc, in_maps, list(range(NC)), trace=trace, tmpdir=tmpdir)
    full = np.empty((B * S, V), np.float32)
    for c in range(NC):
        # core c writes its logits rows rotated by -512c; un-rotate
        full[:, VSH * c:VSH * (c + 1)] = np.roll(res.results[c]["logits"], 512 * c, axis=0)
    return full.reshape(B, S, V), res


def kernel(**inputs) -> np.ndarray:
    out, _ = run(inputs)
    return out


# revision 30
# speedup vs baseline: 1.1726x; 1.0089x over previous
"""MiniGPT forward pass on 8 Trainium2 NeuronCores (Bass/Tile SPMD kernel).

Model: V=32000, T=2048, D=512, H=8 heads, L=4 layers, DFF=2048, B=2, S=2048.

Sharding (8 cores, one SPMD program):
- Tokens: core c owns 512 tokens = flat[512c : 512c+512] (batch c//4).
- Attention: head-parallel; core c computes head c for both batches (batch b
  in partition half 64b) over the batch's full 2048 tokens. QKV and attention
  outputs are redistributed with AllToAll over all 8 cores.
- LM head: vocab-parallel; core c computes logits[:, 4000c:4000c+4000] for all
  4096 tokens after an AllGather of the final hidden states. Logit rows are
  written rotated by -512c (so row offsets are core-independent); the host
  un-rotates with np.roll.

Layouts: residual h is [token, feature] fp32 in SBUF. LN outputs are cast to
bf16 and PE-transposed to [feature, token] as matmul operands. LN gamma/beta
are folded into the following matmul weights on the host. proj/ffn2 biases are
added via rank-1 matmuls into PSUM; qkv bias rides the ACT-engine PSUM->SBUF
copy. The causal mask is applied by skipping fully-masked 128-col strips and
multiplying one 128x128 triangular 0/1 mask after exp on the DVE.
"""
import sys

sys.path.insert(0, "/opt/trn_rl_repo")

import numpy as np
import ml_dtypes

import concourse.bass as bass
import concourse.mybir as mybir
import concourse.tile as tile
from concourse import bacc, bass_utils

BF16 = mybir.dt.bfloat16
F32 = mybir.dt.float32
I32 = mybir.dt.int32
AF = mybir.ActivationFunctionType
OP = mybir.AluOpType

V, T, D, H, L = 32000, 2048, 512, 8, 4
HD = D // H          # 64
DFF = 4 * D          # 2048
B, S = 2, 2048
NC = 8               # cores
TOK = 512            # tokens per core
VSH = V // NC        # 4000 vocab per core


def build_nc(zero_lmb: bool):
    nc = bacc.Bacc("TRN2", target_bir_lowering=False, debug=False, num_devices=NC)

    # ---- I/O ----
    tok_emb = nc.dram_tensor("tok_emb", [V, D], F32, kind="ExternalInput")
    pos = nc.dram_tensor("pos", [TOK, D], F32, kind="ExternalInput")
    xidx = nc.dram_tensor("xidx", [TOK, 1], I32, kind="ExternalInput")
    wqkvT = nc.dram_tensor("wqkvT", [L, D, 3 * D], BF16, kind="ExternalInput")
    bqkv = nc.dram_tensor("bqkv", [L * 12 * 128, 1], F32, kind="ExternalInput")
    wprojT = nc.dram_tensor("wprojT", [L, D, D], BF16, kind="ExternalInput")
    bproj = nc.dram_tensor("bproj", [L, 1, D], BF16, kind="ExternalInput")
    wffn1T = nc.dram_tensor("wffn1T", [L, D, DFF], BF16, kind="ExternalInput")
    bffn1 = nc.dram_tensor("bffn1", [L * 16 * 128, 1], F32, kind="ExternalInput")
    wffn2T = nc.dram_tensor("wffn2T", [L, DFF, D], BF16, kind="ExternalInput")
    bffn2 = nc.dram_tensor("bffn2", [L, 1, D], BF16, kind="ExternalInput")
    lmT = nc.dram_tensor("lmT", [D, VSH], BF16, kind="ExternalInput")
    lmb = nc.dram_tensor("lmb", [1, VSH], F32, kind="ExternalInput")
    ident_in = nc.dram_tensor("ident_in", [128, 128], BF16, kind="ExternalInput")
    ones_in = nc.dram_tensor("ones_in", [1, 128], BF16, kind="ExternalInput")
    tri_in = nc.dram_tensor("tri_in", [128, 128], BF16, kind="ExternalInput")
    logits = nc.dram_tensor("logits", [B * S, VSH], F32, kind="ExternalOutput")

    lmidx = nc.dram_tensor("lmidx", [128, 28], I32, kind="ExternalInput")

    # ---- internal DRAM (collective bounces) ----
    # per-shard rows: 64 k + 64 v + 64 q = 192 (q rides the kv AllToAll)
    kv_ai = [nc.dram_tensor(f"kv_ai{l}", [3 * D, TOK], BF16) for l in range(L)]
    kv_ao = [nc.dram_tensor(f"kv_ao{l}", [3 * D, TOK], BF16) for l in range(L)]
    att_ai = [nc.dram_tensor(f"att_ai{l}", [D, TOK], BF16) for l in range(L)]
    att_ao = [nc.dram_tensor(f"att_ao{l}", [D, TOK], BF16) for l in range(L)]
    ag_in = nc.dram_tensor("ag_in", [D, TOK], BF16)
    ag_out = nc.dram_tensor("ag_out", [NC * D, TOK], BF16, addr_space="Shared")
    grp = [list(range(NC))]

    with tile.TileContext(nc) as tc:
        with (
            tc.tile_pool(name="const", bufs=1) as cp,
            tc.tile_pool(name="persist", bufs=1) as pp,
        ):
            ident = cp.tile([128, 128], BF16, name="ident")
            ones_r = cp.tile([1, 128], BF16, name="ones_r")
            tri = cp.tile([128, 128], BF16, name="tri")
            eps_t = cp.tile([128, 1], F32, name="eps_t")
            if not zero_lmb:
                lmb_bc = cp.tile([128, VSH], BF16, name="lmb_bc")
                brow = cp.tile([1, 512], F32, name="brow")
                brow_bf = cp.tile([1, 512], BF16, name="brow_bf")
            hts = [pp.tile([128, D], F32, name=f"h{t}") for t in range(4)]
            idx_sb = pp.tile([128, 4], I32, name="idx_sb")
            lmidx_sb = pp.tile([128, 28], I32, name="lmidx_sb")
            vones = pp.tile([128, 32 * 65], BF16, name="vones")
            hfT = pp.tile([128, 4 * 512], BF16, name="hfT")

            with (
                tc.tile_pool(name="wpool", bufs=2) as wp,
                tc.tile_pool(name="work", bufs=2) as wk,
                tc.tile_pool(name="exppool", bufs=4) as ep,
                tc.tile_pool(name="pmm", bufs=2, space="PSUM") as pmm,
                tc.tile_pool(name="psc", bufs=3, space="PSUM") as psc,
                tc.tile_pool(name="pout", bufs=3, space="PSUM") as pout,
            ):
                # ================= prologue =================
                nc.sync.dma_start(out=ident[:], in_=ident_in[:])
                nc.sync.dma_start(out=ones_r[:], in_=ones_in[:])
                nc.sync.dma_start(out=tri[:], in_=tri_in[:])
                nc.vector.memset(eps_t[:], 1e-5)
                nc.vector.memset(vones[:], 1.0)

                if not zero_lmb:
                    def bcast_row(dst_ap, src_dram_ap, n):
                        done = 0
                        while done < n:
                            w = min(512, n - done)
                            nc.sync.dma_start(out=brow[:, :w], in_=src_dram_ap[:, done:done + w])
                            nc.vector.tensor_copy(out=brow_bf[:, :w], in_=brow[:, :w])
                            ps = pmm.tile([128, 512], F32, tag="pmm")
                            nc.tensor.matmul(ps[:, :w], lhsT=ones_r[:, :], rhs=brow_bf[:, :w],
                                             start=True, stop=True)
                            nc.vector.tensor_copy(out=dst_ap[:, done:done + w], in_=ps[:, :w])
                            done += w
                    bcast_row(lmb_bc[:, :], lmb[:, :], VSH)

                # embeddings -> residual h [128 tok, 4 blocks * 512 feat] fp32
                for t in range(4):
                    nc.sync.dma_start(out=idx_sb[:, t:t + 1], in_=xidx[128 * t:128 * (t + 1), :])
                nc.sync.dma_start(out=lmidx_sb[:], in_=lmidx[:])
                for t in range(4):
                    emb = wk.tile([128, D], F32, tag="emb", bufs=1)
                    nc.gpsimd.indirect_dma_start(
                        out=emb[:], out_offset=None, in_=tok_emb[:],
                        in_offset=bass.IndirectOffsetOnAxis(ap=idx_sb[:, t:t + 1], axis=0),
                    )
                    pos_t = wk.tile([128, D], F32, tag="emb2", bufs=1)
                    nc.sync.dma_start(out=pos_t[:], in_=pos[128 * t:128 * (t + 1), :])
                    nc.vector.tensor_tensor(out=hts[t][:], in0=emb[:], in1=pos_t[:], op=OP.add)

                # ---- helpers ----
                def layernorm_T(srcs, dst_bf_T):
                    """srcs: 4 tiles [128, D] fp32 [tok, feat]. Writes dst_bf_T
                    [128, 4*512] bf16 = transposed ([feat-ptile, tok]) normalized."""
                    hln = wk.tile([128, 4 * D], BF16, tag="hln", bufs=1)
                    nmu4 = wk.tile([128, 4], F32, tag="lnmu")
                    s4 = wk.tile([128, 4], F32, tag="lns4")
                    ssq4 = wk.tile([128, 4], F32, tag="lnssq")
                    mu2 = wk.tile([128, 4], F32, tag="lnmu2")
                    var4 = wk.tile([128, 4], F32, tag="lnvar")
                    rs4 = wk.tile([128, 4], F32, tag="lnrs")
                    for t in range(4):
                        sq = wk.tile([128, D], F32, tag="lnsq", bufs=2)
                        nc.vector.tensor_reduce(out=s4[:, t:t + 1], in_=srcs[t][:],
                                                axis=mybir.AxisListType.X, op=OP.add)
                        nc.scalar.activation(out=sq[:], in_=srcs[t][:], func=AF.Square,
                                             accum_out=ssq4[:, t:t + 1])
                    nc.vector.tensor_scalar_mul(nmu4[:], s4[:], -1.0 / D)
                    nc.vector.tensor_tensor(out=mu2[:], in0=nmu4[:], in1=nmu4[:], op=OP.mult)
                    nc.vector.scalar_tensor_tensor(out=var4[:], in0=ssq4[:], scalar=1.0 / D,
                                                   in1=mu2[:], op0=OP.mult, op1=OP.subtract)
                    nc.scalar.activation(out=rs4[:], in_=var4[:], func=AF.Ln, bias=eps_t[:])
                    nc.scalar.activation(out=rs4[:], in_=rs4[:], func=AF.Exp, scale=-0.5)
                    for t in range(4):
                        nc.vector.tensor_scalar(out=hln[:, D * t:D * (t + 1)], in0=srcs[t][:],
                                                scalar1=nmu4[:, t:t + 1], scalar2=rs4[:, t:t + 1],
                                                op0=OP.add, op1=OP.mult)
                    for f in range(4):
                        tp = pmm.tile([128, 512], BF16, tag="pmm")
                        for t in range(4):
                            nc.tensor.transpose(out=tp[:, 128 * t:128 * (t + 1)],
                                                in_=hln[:, D * t + 128 * f: D * t + 128 * (f + 1)],
                                                identity=ident[:])
                        nc.vector.tensor_copy(out=dst_bf_T[:, 512 * f:512 * (f + 1)], in_=tp[:])

                # ================= transformer layers =================
                for l in range(L):
                    wq = wp.tile([128, 4 * 1536], BF16, tag="wq")
                    nc.sync.dma_start(out=wq[:].rearrange("p (c e) -> p c e", c=4),
                                      in_=wqkvT[l].rearrange("(c p) e -> p c e", p=128))
                    wpj = wp.tile([128, 4 * 512], BF16, tag="wpj")
                    nc.sync.dma_start(out=wpj[:].rearrange("p (c e) -> p c e", c=4),
                                      in_=wprojT[l].rearrange("(c p) e -> p c e", p=128))
                    wf1 = wp.tile([128, 4 * 2048], BF16, tag="wf1")
                    nc.sync.dma_start(out=wf1[:].rearrange("p (c e) -> p c e", c=4),
                                      in_=wffn1T[l].rearrange("(c p) e -> p c e", p=128))
                    wf2 = wp.tile([128, 16 * 512], BF16, tag="wf2")
                    nc.sync.dma_start(out=wf2[:].rearrange("p (c e) -> p c e", c=16),
                                      in_=wffn2T[l].rearrange("(c p) e -> p c e", p=128))
                    bq = wp.tile([128, 12], F32, tag="bq")
                    nc.sync.dma_start(out=bq[:],
                                      in_=bqkv[l * 1536:(l + 1) * 1536, :].rearrange(
                                          "(o p) x -> p (o x)", p=128))
                    bf1 = wp.tile([128, 16], F32, tag="bf1")
                    nc.sync.dma_start(out=bf1[:],
                                      in_=bffn1[l * 2048:(l + 1) * 2048, :].rearrange(
                                          "(o p) x -> p (o x)", p=128))
                    bprow = wp.tile([1, 512], BF16, tag="bprow")
                    nc.sync.dma_start(out=bprow[:], in_=bproj[l])
                    bf2row = wp.tile([1, 512], BF16, tag="bf2row")
                    nc.sync.dma_start(out=bf2row[:], in_=bffn2[l])

                    # -- LN1 + transpose --
                    hlnT = wk.tile([128, 4 * 512], BF16, tag="hlnT", bufs=1)
                    layernorm_T(hts, hlnT)

                    # -- qkvT = W' @ hlnT ([3D feat, 512 tok]); k,v first, bounce
                    # DMAs issued per-block so the kv AllToAll fires early --
                    qkvT = wk.tile([128, 12 * 512], BF16, tag="qkvT", bufs=1)
                    for o in [4, 5, 6, 7, 8, 9, 10, 11, 0, 1, 2, 3]:
                        ps = pmm.tile([128, 512], F32, tag="pmm")
                        for kc in range(4):
                            nc.tensor.matmul(ps[:],
                                             lhsT=wq[:, 1536 * kc + 128 * o:1536 * kc + 128 * (o + 1)],
                                             rhs=hlnT[:, 512 * kc:512 * (kc + 1)],
                                             start=(kc == 0), stop=(kc == 3))
                        nc.scalar.activation(out=qkvT[:, 512 * o:512 * (o + 1)], in_=ps[:],
                                             func=AF.Identity, bias=bq[:, o:o + 1])
                        if 4 <= o < 8:        # k block: shards s = 2(o-4)+hh
                            for hh in (0, 1):
                                s_ = 2 * (o - 4) + hh
                                nc.sync.dma_start(out=kv_ai[l][192 * s_:192 * s_ + 64, :],
                                                  in_=qkvT[64 * hh:64 * hh + 64, 512 * o:512 * (o + 1)])
                        elif o >= 8:          # v block
                            for hh in (0, 1):
                                s_ = 2 * (o - 8) + hh
                                nc.sync.dma_start(out=kv_ai[l][192 * s_ + 64:192 * s_ + 128, :],
                                                  in_=qkvT[64 * hh:64 * hh + 64, 512 * o:512 * (o + 1)])
                        else:                 # q block
                            for hh in (0, 1):
                                s_ = 2 * o + hh
                                nc.sync.dma_start(out=kv_ai[l][192 * s_ + 128:192 * s_ + 192, :],
                                                  in_=qkvT[64 * hh:64 * hh + 64, 512 * o:512 * (o + 1)])
                            if o == 3:
                                nc.gpsimd.collective_compute(
                                    "AllToAll", OP.bypass, replica_groups=grp,
                                    ins=[kv_ai[l][:]], outs=[kv_ao[l][:]],
                                )
                    qT = wk.tile([128, 2048], BF16, tag="qT", bufs=1)
                    kT = wk.tile([128, 2048], BF16, tag="kT", bufs=1)
                    vT = wk.tile([128, 2048], BF16, tag="vT", bufs=1)
                    for r in range(8):
                        b_, rr = r // 4, r % 4
                        nc.sync.dma_start(out=kT[64 * b_:64 * b_ + 64, 512 * rr:512 * (rr + 1)],
                                          in_=kv_ao[l][192 * r:192 * r + 64, :])
                        nc.sync.dma_start(out=vT[64 * b_:64 * b_ + 64, 512 * rr:512 * (rr + 1)],
                                          in_=kv_ao[l][192 * r + 64:192 * r + 128, :])
                        nc.sync.dma_start(out=qT[64 * b_:64 * b_ + 64, 512 * rr:512 * (rr + 1)],
                                          in_=kv_ao[l][192 * r + 128:192 * r + 192, :])
                    for h2 in range(2):
                        hb = 64 * h2
                        for i in range(16):
                            tp = pmm.tile([128, 64], BF16, tag="pmm")
                            nc.tensor.transpose(out=tp[:], in_=vT[hb:hb + 64, 128 * i:128 * (i + 1)],
                                                identity=ident[hb:hb + 64, hb:hb + 64])
                            nc.vector.tensor_copy(out=vones[:, 65 * (16 * h2 + i):65 * (16 * h2 + i) + 64],
                                                  in_=tp[:])

                    # -- attention (my head, both batches, causal); score of item
                    # t+1 issues before AV of item t so PE never waits on exp --
                    attnT = wk.tile([64, 4096], BF16, tag="attnT", bufs=1)
                    for h2 in range(2):
                        hb = 64 * h2
                        for p in range(2):
                            outp = [pout.tile([65, 512], F32, tag="pout", name=f"outp{jl}")
                                    for jl in range(2)]
                            items = []
                            for i in range(8 * p + 8):
                                jlmin = max(0, i // 4 - 2 * p)
                                for jl in (0, 1):
                                    if jl < jlmin:
                                        continue
                                    diag = (i // 4 == 2 * p + jl)
                                    c0 = 128 * (i % 4) if diag else 0
                                    items.append((i, jl, diag, c0))

                            def score(it):
                                i, jl, diag, c0 = it
                                sc = psc.tile([128, 512], F32, tag="psc")
                                nc.tensor.matmul(
                                    sc[:, c0:512],
                                    lhsT=kT[hb:hb + 64, 128 * i:128 * (i + 1)],
                                    rhs=qT[hb:hb + 64,
                                           1024 * p + 512 * jl + c0:1024 * p + 512 * (jl + 1)],
                                    start=True, stop=not diag)
                                if diag:
                                    # additive -1e9 upper-strip mask, 128 cols on the PE
                                    nc.tensor.matmul(sc[:, c0:c0 + 128], lhsT=ident[:],
                                                     rhs=tri[:], start=False, stop=True)
                                ex = ep.tile([128, 512], BF16, tag="ex")
                                nc.scalar.activation(out=ex[:, c0:512], in_=sc[:, c0:512],
                                                     func=AF.Exp, scale=float(HD) ** -0.5)
                                return ex

                            def av(it, ex):
                                i, jl, diag, c0 = it
                                kmax = 4 * (2 * p + jl) + 3
                                nc.tensor.matmul(
                                    outp[jl][:, c0:512],
                                    lhsT=vones[:, 65 * (16 * h2 + i):65 * (16 * h2 + i + 1)],
                                    rhs=ex[:, c0:512],
                                    start=(i == 0), stop=(i == kmax))

                            pend = []
                            for it in items:
                                ex = score(it)
                                pend.append((it, ex))
                                if len(pend) > 2:
                                    av(*pend.pop(0))
                            while pend:
                                av(*pend.pop(0))

                            # normalize: rows 0..63 /= row 64
                            for jl in range(2):
                                dnb = wk.tile([1, 512], BF16, tag="rcb", bufs=2)
                                nc.vector.tensor_copy(out=dnb[:], in_=outp[jl][64:65, :])
                                bc = psc.tile([64, 512], F32, tag="psc")
                                nc.tensor.matmul(bc[:], lhsT=ones_r[:, 0:64],
                                                 rhs=dnb[:], start=True, stop=True)
                                rcs = wk.tile([64, 512], F32, tag="bcs", bufs=2)
                                nc.vector.reciprocal_approx_fast(out=rcs[:], in_=bc[:])
                                nc.vector.tensor_tensor(
                                    out=attnT[:, 2048 * h2 + 1024 * p + 512 * jl:2048 * h2 + 1024 * p + 512 * (jl + 1)],
                                    in0=outp[jl][0:64, :], in1=rcs[:], op=OP.mult)
                    # A2A attention outputs back to token owners
                    for s_ in range(8):
                        nc.sync.dma_start(out=att_ai[l][64 * s_:64 * (s_ + 1), :],
                                          in_=attnT[:, 512 * s_:512 * (s_ + 1)])
                    nc.gpsimd.collective_compute(
                        "AllToAll", OP.bypass, replica_groups=grp,
                        ins=[att_ai[l][:]], outs=[att_ao[l][:]],
                    )
                    aT = wk.tile([128, 4 * 512], BF16, tag="aT", bufs=1)
                    for r in range(4):
                        nc.sync.dma_start(out=aT[:, 512 * r:512 * (r + 1)],
                                          in_=att_ao[l][128 * r:128 * (r + 1), :])

                    # -- proj + residual (bias via rank-1 matmul) --
                    for t in range(4):
                        ps = pmm.tile([128, 512], F32, tag="pmm")
                        for fc in range(4):
                            nc.tensor.matmul(ps[:],
                                             lhsT=aT[:, 512 * fc + 128 * t:512 * fc + 128 * (t + 1)],
                                             rhs=wpj[:, 512 * fc:512 * (fc + 1)],
                                             start=(fc == 0), stop=False)
                        nc.tensor.matmul(ps[:], lhsT=ones_r[:, 0:128], rhs=bprow[:],
                                         start=False, stop=True)
                        nc.vector.tensor_tensor(out=hts[t][:], in0=hts[t][:], in1=ps[:], op=OP.add)

                    # -- LN2 + FFN; ffn2 tiles t0/t1 accumulate in lockstep with
                    # gelu output so the PE never waits for the gelu tail --
                    hln2T = wk.tile([128, 4 * 512], BF16, tag="hlnT", bufs=1)
                    layernorm_T(hts, hln2T)
                    fT = wk.tile([128, 16 * 512], BF16, tag="fT", bufs=1)
                    psA = psc.tile([128, 512], F32, tag="psc")
                    psB = psc.tile([128, 512], F32, tag="psc")
                    for o in range(16):
                        ps = pmm.tile([128, 512], F32, tag="pmm")
                        for kc in range(4):
                            nc.tensor.matmul(ps[:],
                                             lhsT=wf1[:, 2048 * kc + 128 * o:2048 * kc + 128 * (o + 1)],
                                             rhs=hln2T[:, 512 * kc:512 * (kc + 1)],
                                             start=(kc == 0), stop=(kc == 3))
                        nc.scalar.activation(out=fT[:, 512 * o:512 * (o + 1)], in_=ps[:],
                                             func=AF.Gelu, bias=bf1[:, o:o + 1])
                        for t, pst in ((0, psA), (1, psB)):
                            nc.tensor.matmul(pst[:],
                                             lhsT=fT[:, 512 * o + 128 * t:512 * o + 128 * (t + 1)],
                                             rhs=wf2[:, 512 * o:512 * (o + 1)],
                                             start=(o == 0), stop=False)
                    for t, pst in ((0, psA), (1, psB)):
                        nc.tensor.matmul(pst[:], lhsT=ones_r[:, 0:128], rhs=bf2row[:],
                                         start=False, stop=True)
                        nc.vector.tensor_tensor(out=hts[t][:], in0=hts[t][:], in1=pst[:], op=OP.add)
                    for t in (2, 3):
                        ps = pmm.tile([128, 512], F32, tag="pmm")
                        for kc in range(16):
                            nc.tensor.matmul(ps[:],
                                             lhsT=fT[:, 512 * kc + 128 * t:512 * kc + 128 * (t + 1)],
                                             rhs=wf2[:, 512 * kc:512 * (kc + 1)],
                                             start=(kc == 0), stop=False)
                        nc.tensor.matmul(ps[:], lhsT=ones_r[:, 0:128], rhs=bf2row[:],
                                         start=False, stop=True)
                        nc.vector.tensor_tensor(out=hts[t][:], in0=hts[t][:], in1=ps[:], op=OP.add)

                # ================= final LN + AllGather =================
                layernorm_T(hts, hfT)
                for f in range(4):
                    nc.sync.dma_start(out=ag_in[128 * f:128 * (f + 1), :],
                                      in_=hfT[:, 512 * f:512 * (f + 1)])
                nc.gpsimd.collective_compute(
                    "AllGather", OP.bypass, replica_groups=[list(range(NC))],
                    ins=[ag_in[:]], outs=[ag_out[:]],
                )

            # ================= LM head (body pools closed, PSUM free) ========
            with (
                tc.tile_pool(name="lmw", bufs=1) as lw,
                tc.tile_pool(name="lmwork", bufs=3) as lk,
                tc.tile_pool(name="lmgat", bufs=2) as lg,
                tc.tile_pool(name="plm", bufs=2, space="PSUM") as plm,
            ):
                lmw = lw.tile([128, 4 * VSH], BF16, name="lmw")
                nc.sync.dma_start(out=lmw[:].rearrange("p (c e) -> p c e", c=4),
                                  in_=lmT[:].rearrange("(c p) e -> p c e", p=128))

                def lm_tile(lhs_slices, out_row):
                    stage = lk.tile([128, VSH], F32, tag="stage")
                    for hf in range(2):
                        ps = plm.tile([128, 2048], F32, tag="plm")
                        for kc in range(4):
                            for vc in range(4):
                                w0 = VSH * kc + 2000 * hf + 500 * vc
                                nc.tensor.matmul(
                                    ps[:, 512 * vc:512 * vc + 500],
                                    lhsT=lhs_slices[kc],
                                    rhs=lmw[:, w0:w0 + 500],
                                    start=(kc == 0), stop=(kc == 3))
                        ps3 = ps[:].rearrange("p (b e) -> p b e", b=4)
                        st3 = stage[:, 2000 * hf:2000 * (hf + 1)].rearrange("p (b e) -> p b e", b=4)
                        if zero_lmb:
                            nc.vector.tensor_copy(out=st3[:, 0:2, :], in_=ps3[:, 0:2, 0:500])
                            nc.scalar.copy(out=st3[:, 2:4, :], in_=ps3[:, 2:4, 0:500])
                        else:
                            bc3 = lmb_bc[:, 2000 * hf:2000 * (hf + 1)].rearrange("p (b e) -> p b e", b=4)
                            nc.vector.tensor_tensor(out=st3, in0=ps3[:, :, 0:500], in1=bc3, op=OP.add)
                    nc.sync.dma_start(out=logits[out_row:out_row + 128, :], in_=stage[:])

                # 4 local token tiles first (read hfT directly; overlaps the AllGather)
                for u in range(4):
                    lm_tile([hfT[:, 512 * kc + 128 * u:512 * kc + 128 * (u + 1)] for kc in range(4)],
                            128 * u)
                # 7 remote ranks (rotated order (c+1+k)%8); one gather per (rank, kc)
                for k in range(7):
                    rksb = lg.tile([128, 4 * 512], BF16, tag="rk")
                    for kc in range(4):
                        nc.gpsimd.indirect_dma_start(
                            out=rksb[:, 512 * kc:512 * (kc + 1)], out_offset=None,
                            in_=ag_out[:],
                            in_offset=bass.IndirectOffsetOnAxis(
                                ap=lmidx_sb[:, 4 * k + kc:4 * k + kc + 1], axis=0),
                        )
                    for u in range(4):
                        lm_tile([rksb[:, 512 * kc + 128 * u:512 * kc + 128 * (u + 1)]
                                 for kc in range(4)],
                                512 * (k + 1) + 128 * u)

    nc.compile()
    return nc


_NC_CACHE = {}


def _get_nc(zero_lmb: bool = True):
    if zero_lmb not in _NC_CACHE:
        _NC_CACHE[zero_lmb] = build_nc(zero_lmb)
    return _NC_CACHE[zero_lmb]


def _prep_inputs(inputs):
    bf = ml_dtypes.bfloat16
    tok_emb = np.asarray(inputs["tok_emb"], np.float32)
    pos_emb = np.asarray(inputs["pos_emb"], np.float32)
    x = np.asarray(inputs["x"]).astype(np.int32).reshape(-1)  # [4096] flat

    def eff(w, g, b, wb):
        # fold the preceding layernorm's gamma/beta into w (out,in) and bias
        w = np.asarray(w, np.float32)
        weff = w * np.asarray(g, np.float32)[None, :]
        beff = w @ np.asarray(b, np.float32) + np.asarray(wb, np.float32)
        return weff, beff

    wqkvT = np.zeros((L, D, 3 * D), bf)
    bqkv = np.zeros((L, 12, 128), np.float32)
    wprojT = np.zeros((L, D, D), bf)
    bproj = np.zeros((L, 1, D), bf)
    wffn1T = np.zeros((L, D, DFF), bf)
    bffn1 = np.zeros((L, 16, 128), np.float32)
    wffn2T = np.zeros((L, DFF, D), bf)
    bffn2 = np.zeros((L, 1, D), bf)
    for l in range(L):
        w, b = eff(inputs["qkv_w"][l], inputs["ln1_g"][l], inputs["ln1_b"][l], inputs["qkv_b"][l])
        wqkvT[l] = w.T.astype(bf)
        bqkv[l] = b.reshape(12, 128)
        wprojT[l] = np.asarray(inputs["proj_w"][l], np.float32).T.astype(bf)
        bproj[l, 0] = np.asarray(inputs["proj_b"][l], np.float32).astype(bf)
        w, b = eff(inputs["ffn1_w"][l], inputs["ln2_g"][l], inputs["ln2_b"][l], inputs["ffn1_b"][l])
        wffn1T[l] = w.T.astype(bf)
        bffn1[l] = b.reshape(16, 128)
        wffn2T[l] = np.asarray(inputs["ffn2_w"][l], np.float32).T.astype(bf)
        bffn2[l, 0] = np.asarray(inputs["ffn2_b"][l], np.float32).astype(bf)
    lmw, lmbf = eff(inputs["lm_w"], inputs["lnf_g"], inputs["lnf_b"], inputs["lm_b"])
    zero_lmb = not np.any(lmbf)

    # additive causal mask for the 128-wide diagonal strip: 0 keep, -1e9 drop
    tri_m = np.where(np.arange(128)[:, None] <= np.arange(128)[None, :],
                     0.0, -1.0e9).astype(bf)

    common = dict(tok_emb=tok_emb, wqkvT=wqkvT, bqkv=bqkv.reshape(L * 12 * 128, 1),
                  wprojT=wprojT, bproj=bproj, wffn1T=wffn1T,
                  bffn1=bffn1.reshape(L * 16 * 128, 1), wffn2T=wffn2T, bffn2=bffn2,
                  tri_in=tri_m, ident_in=np.eye(128, dtype=bf),
                  ones_in=np.ones((1, 128), bf))
    in_maps = []
    pvec = np.arange(128, dtype=np.int32)
    for c in range(NC):
        s0 = 512 * (c % 4)
        m = dict(common)
        m["pos"] = pos_emb[s0:s0 + 512]
        m["xidx"] = x[512 * c:512 * (c + 1)].reshape(TOK, 1)
        lmidx = np.zeros((128, 28), np.int32)
        for k in range(7):
            rk = (c + 1 + k) % NC
            for kc in range(4):
                lmidx[:, 4 * k + kc] = 512 * rk + 128 * kc + pvec  # gather rows
        m["lmidx"] = lmidx
        m["lmT"] = np.ascontiguousarray(lmw[VSH * c:VSH * (c + 1)].T.astype(bf))
        m["lmb"] = lmbf[VSH * c:VSH * (c + 1)].reshape(1, VSH).copy()
        in_maps.append(m)
    return in_maps, zero_lmb


def run(inputs, trace=False, tmpdir=None):
    in_maps, zero_lmb = _prep_inputs(inputs)
    nc = _get_nc(zero_lmb)
    res = bass_utils.run_bass_kernel_spmd(nc, in_maps, list(range(NC)), trace=trace, tmpdir=tmpdir)
    full = np.empty((B * S, V), np.float32)
    for c in range(NC):
        # core c writes its logits rows rotated by -512c; un-rotate
        full[:, VSH * c:VSH * (c + 1)] = np.roll(res.results[c]["logits"], 512 * c, axis=0)
    return full.reshape(B, S, V), res


def kernel(**inputs) -> np.ndarray:
    out, _ = run(inputs)
    return out
